# revision 1
# baseline (speedup 1.0000x reference)
"""IoU loss kernel for Trainium2, data-parallel over 8 NeuronCores.

Math (per box, columns = x-center, y-center, half-size s):
    w = relu(min(x+s, x'+s') - max(x-s, x'-s'))
      = relu((s+s') - max(|x-x'|, |s-s'|))          # S - max identity
    h likewise with y.
    overlap = w*h
    union   = 4s^2 + 4s'^2 - overlap = 2(S^2 + D^2) - overlap,
              S = s+s', D = s-s'
    iou     = overlap / (union + 1e-7)
    loss    = -sum(log(iou + 1e-7));  iou_sum = sum(iou)

The DMA stream (two fp32 loads per tile, 24 MiB/core total) is the
roofline: ~76us at the 360 GB/s per-core HBM rate. Everything else is
scheduled under its shadow:

  DVE  (~7.5us/KTile): dx, dy, S, D (fp32 strided reads -> fp16, 1x
         mode), mw/mh maxes (fp16, 2x mode), relus (tensor_scalar, 4x
         mode), r = 1/(u+eps) via reciprocal_approx_fast (~18 bits).
  ACT  (~7.6us/KTile): |dx|, |dy|, |D| (Abs), 2S^2/2D^2 (Square with
         scale=sqrt2), Ln(iou+eps) accum -> loss partial, Copy(iou)
         accum -> iou partial. All funcs live in the `natural_log`
         table set -> ONE table load total (Exp would force a 1.3us
         table swap per use - hence reciprocal on DVE instead of
         Exp(-Ln(u))).
  Pool (~5us/KTile): hr = S - mh, ov = w*h, u' = q12 - ov, 5/8 of
         iou = ov*r (tensor_tensor only - the sole elementwise opcode
         with Q7 ucode; tensor_scalar/STT/TTR compile but crash the
         Q7 dispatcher at runtime, and TT max fails the ISA check).
  PE   : drains that absorb the multi-condition raw-slot-recycle waits
         (WAR + WAW + DMA lane) so the loads stay wait-free; a drain on
         the SP queue itself would stall the DMA stream ~1.7us/tile.

The per-segment work is software-pipelined with a one-segment skew
(li/ic two segments) so no engine's in-order queue ever head-of-line
blocks on the tail of the previous segment's dependency chain; the
trailing segments shrink (512/256/128/128) to cut the drain-out tail.
Host: final [128, 2*NSEG] x 8 cores partial-sum reduction in float64.
"""

import numpy as np

import concourse.bass as bass
import concourse.mybir as mybir
from concourse import tile
from concourse.bass_utils import run_bass_kernel_spmd

N = 8388608
NCORES = 8
NS = N // NCORES  # 1048576 boxes per core
P = 128
W = 1024          # boxes per partition per full tile
T = NS // (P * W)  # 8 full-tile units per core
EPS = 1e-7
RT2 = 1.4142135623730951

F32 = mybir.dt.float32
F16 = mybir.dt.float16
Op = mybir.AluOpType
Act = mybir.ActivationFunctionType


def _build(T_: int = T, W_: int = W, compile_passes: bool = True) -> bass.Bass:
    from concourse import bacc
    from concourse.tile_rust import add_dep_helper

    # Tile widths: the DMA stream is gapless, so span = stream + drain-out
    # tail. Progressively smaller trailing tiles shrink both the engine
    # queue backlog when the last load lands and the final serial chain.
    # The last three tiles use a dedicated small raw-slot tag so their
    # loads never wait on big-slot recycling.
    segs = ([W_] * (T_ - 1)
            + [W_ // 2, W_ // 2])
    assert sum(segs) == T_ * W_
    NSEG = len(segs)
    SMALLW = W_ // 2
    MICROW = W_ // 8
    NBIGPS = T_ - 1  # 1024-wide segs feed the PE/PSUM iou reduction

    ns = P * W_ * T_
    nc = bacc.Bacc()
    outs_d = nc.dram_tensor("outputs", [ns, 3], F32, kind="ExternalInput")
    tars_d = nc.dram_tensor("targets", [ns, 3], F32, kind="ExternalInput")
    acc_d = nc.dram_tensor("acc", [P, 2 * NSEG], F32, kind="ExternalOutput")
    iouv_d = nc.dram_tensor("iouv", [1, 1024], F32, kind="ExternalOutput")

    offs = [0]
    for w in segs:
        offs.append(offs[-1] + w)

    def seg_view(dram, s):
        b0 = P * offs[s]
        return dram[b0 : b0 + P * segs[s], :].rearrange(
            "(p w) c -> p (w c)", p=P, w=segs[s]
        )

    RAWBUFS = 3

    with tile.TileContext(nc) as tc:
        with (tc.tile_pool(name="main", bufs=2) as pool,
              tc.tile_pool(name="psum", bufs=1,
                           space=bass.MemorySpace.PSUM) as psum):
            accs = pool.tile([P, 2 * NSEG], F32, tag="accs", bufs=1)
            # big segs' iou partials live in PSUM, so their accs columns in
            # the second half are never written; zero them for the store
            nc.vector.memset(accs[:, :], 0.0)
            eps_t = pool.tile([P, 1], F32, tag="eps", bufs=1)
            nc.vector.memset(eps_t[:, :], EPS)
            ones = pool.tile([P, 1], F16, tag="ones", bufs=1)
            nc.vector.memset(ones[:, :], 1.0)
            # PSUM accumulators for the iou partition-sums: group A covers
            # segs 0..GSPLIT-1, group B the rest; each seg contributes two
            # half-width matmuls (PSUM bank = 512 fp32 columns).
            psA1 = psum.tile([1, 512], F32, tag="psA1", bufs=1)
            psA2 = psum.tile([1, 512], F32, tag="psA2", bufs=1)

            lastrd: list = []
            dmaO_h: list = []
            dmaT_h: list = []
            big_idx: list = []
            C: list = []

            def front(t):
                w = segs[t]
                if w <= MICROW:
                    rawO = pool.tile([P, 3 * MICROW], F32, tag="rawOm", bufs=3)
                    rawT = pool.tile([P, 3 * MICROW], F32, tag="rawTm", bufs=3)
                    recycle = None
                elif w <= SMALLW:
                    rawO = pool.tile([P, 3 * SMALLW], F32, tag="rawOs", bufs=3)
                    rawT = pool.tile([P, 3 * SMALLW], F32, tag="rawTs", bufs=3)
                    recycle = None
                else:
                    rawO = pool.tile([P, 3 * W_], F32, tag="rawO", bufs=RAWBUFS)
                    rawT = pool.tile([P, 3 * W_], F32, tag="rawT", bufs=RAWBUFS)
                    nbig = len(big_idx)
                    recycle = big_idx[nbig - RAWBUFS] if nbig >= RAWBUFS else None
                    big_idx.append(t)
                deps = []
                if recycle is not None:
                    dr = nc.tensor.drain(fusable=False)
                    add_dep_helper(dr.ins, lastrd[recycle].ins, sync=True,
                                   reason="absorb DVE WAR tick")
                    add_dep_helper(dr.ins, dmaO_h[recycle].ins, sync=True,
                                   reason="absorb old rawO DMA lane")
                    add_dep_helper(dr.ins, dmaT_h[recycle].ins, sync=True,
                                   reason="absorb old rawT DMA lane")
                    deps = [dr]
                dmaO = nc.sync.dma_start(out=rawO[:, : 3 * w], in_=seg_view(outs_d, t))
                dmaT = nc.sync.dma_start(out=rawT[:, : 3 * w], in_=seg_view(tars_d, t))
                for d in deps:
                    add_dep_helper(dmaO.ins, d.ins, sync=True,
                                   reason="slot guarded by PE drain")
                    add_dep_helper(dmaT.ins, d.ins, sync=True,
                                   reason="slot guarded by PE drain")
                dmaO_h.append(dmaO)
                dmaT_h.append(dmaT)

                # dx below depends on BOTH input DMAs but has one sync-wait
                # slot; absorb rawT's semaphore with a tiny copy.
                dummy = pool.tile([P, 1], F32, tag="dummy")
                nc.vector.tensor_copy(dummy[:, :], rawT[:, 0:1])
                o3 = rawO[:, : 3 * w].rearrange("p (w c) -> p w c", c=3)
                t3 = rawT[:, : 3 * w].rearrange("p (w c) -> p w c", c=3)
                x1, y1, s1 = o3[:, :, 0], o3[:, :, 1], o3[:, :, 2]
                x2, y2, s2 = t3[:, :, 0], t3[:, :, 1], t3[:, :, 2]

                c = {"t": t, "w": w}
                # tags whose only readers run in the same pipeline iteration
                # as the writer get a single buffer (the next generation's
                # WAR lands a full iteration later).
                for nm in ("dx", "dy", "D", "mw", "mh"):
                    c[nm] = pool.tile([P, W_], F16, tag=nm, bufs=1,
                                      name=f"{nm}_{t}")
                for nm in ("S", "adx", "ady", "aD", "qS", "qD", "wr", "hr",
                           "rh", "q12", "ue16", "ov", "iou", "sc"):
                    c[nm] = pool.tile([P, W_], F16, tag=nm, name=f"{nm}_{t}")
                for nm in ("ue32", "r"):
                    c[nm] = pool.tile([P, W_], F32, tag=nm, name=f"{nm}_{t}")

                # DVE: dx first so ACT's Abs chain starts as early as
                # possible (ACT otherwise idles ~0.9us/seg waiting for it)
                nc.vector.tensor_tensor(c["dx"][:, :w], x1, x2, Op.subtract)
                c["ins"] = (x1, y1, s1, x2, y2, s2)
                C.append(c)

            def front2(c):
                # DVE: remaining strided fp32 input ops (1x mode)
                x1, y1, s1, x2, y2, s2 = c["ins"]
                w = c["w"]
                nc.vector.tensor_tensor(c["dy"][:, :w], y1, y2, Op.subtract)
                nc.vector.tensor_tensor(c["S"][:, :w], s1, s2, Op.add)
                lastrd.append(
                    nc.vector.tensor_tensor(c["D"][:, :w], s1, s2, Op.subtract))

                # ACT: abs values and scaled squares (dtype/stride-blind)
                nc.scalar.activation(c["adx"][:, :w], c["dx"][:, :w], Act.Abs)
                nc.scalar.activation(c["ady"][:, :w], c["dy"][:, :w], Act.Abs)
                nc.scalar.activation(c["aD"][:, :w], c["D"][:, :w], Act.Abs)
                nc.scalar.activation(c["qS"][:, :w], c["S"][:, :w], Act.Square,
                                     scale=RT2)
                nc.scalar.activation(c["qD"][:, :w], c["D"][:, :w], Act.Square,
                                     scale=RT2)

            def maxes(c):  # DVE: thresholds, w-extent (relu in place),
                # union precursor. The h-extent runs on Pool/ACT instead.
                w = c["w"]
                nc.vector.tensor_tensor(c["mw"][:, :w], c["adx"][:, :w],
                                        c["aD"][:, :w], Op.max)
                nc.vector.tensor_tensor(c["mh"][:, :w], c["ady"][:, :w],
                                        c["aD"][:, :w], Op.max)
                nc.vector.tensor_sub(c["wr"][:, :w], c["S"][:, :w], c["mw"][:, :w])
                nc.vector.tensor_scalar_max(c["wr"][:, :w], c["wr"][:, :w], 0.0)
                nc.vector.tensor_add(c["q12"][:, :w], c["qS"][:, :w], c["qD"][:, :w])

            def hrst(c):  # Pool: hr = S - mh (Q7 ucode exists for TT only)
                w = c["w"]
                nc.gpsimd.tensor_tensor(c["hr"][:, :w], c["S"][:, :w],
                                        c["mh"][:, :w], Op.subtract)

            def rhst(c):  # DVE: relu of the h-extent (tensor_scalar, 4x
                # mode; ACT was running at stream-level occupancy)
                w = c["w"]
                nc.vector.tensor_scalar_max(c["rh"][:, :w], c["hr"][:, :w], 0.0)

            def recip(c):  # DVE: r = 1/(u+eps), ~18 bits, plenty vs 2e-2
                w = c["w"]
                nc.vector.reciprocal_approx_fast(c["r"][:, :w], c["ue32"][:, :w])

            def unionst(c):  # Pool: ov = w*h, ue16 = q12 - ov
                w = c["w"]
                nc.gpsimd.tensor_tensor(c["ov"][:, :w], c["wr"][:, :w],
                                        c["rh"][:, :w], Op.mult)
                nc.gpsimd.tensor_tensor(c["ue16"][:, :w], c["q12"][:, :w],
                                        c["ov"][:, :w], Op.subtract)



            def ioust_dve(c):
                # DVE: 3/8 of iou = ov * r (1x-mode op, so DVE takes the
                # smaller share; the Q7 two-input floor caps Pool). The tiny
                # trailing segs compute the whole product here; their iou
                # partial is accumulated by an ACT Copy in accum().
                t, w = c["t"], c["w"]
                if t >= NBIGPS:
                    nc.vector.tensor_mul(c["iou"][:, :w], c["ov"][:, :w],
                                         c["r"][:, :w])
                    return
                h = w // 4
                nc.vector.tensor_mul(c["iou"][:, :h], c["ov"][:, :h],
                                     c["r"][:, :h])

            def ioust_pool(c):  # Pool: remaining 3/4 of iou = ov * r
                w, h = c["w"], c["w"] // 4
                if c["t"] >= NBIGPS:
                    return
                nc.gpsimd.tensor_tensor(c["iou"][:, h:w], c["ov"][:, h:w],
                                        c["r"][:, h:w], Op.mult)

            def iou_psum(c):  # PE: per-seg partition-sums into PSUM banks
                t, w, h = c["t"], c["w"], c["w"] // 2
                if t >= NBIGPS:
                    return
                first = t == 0
                last = t == NBIGPS - 1
                nc.tensor.matmul(psA1[:, :h], ones[:, :], c["iou"][:, :h],
                                 start=first, stop=last)
                nc.tensor.matmul(psA2[:, :h], ones[:, :],
                                 c["iou"][:, h : 2 * h],
                                 start=first, stop=last)

            def ue32st(c):  # ACT: fp32 upcast with the eps floor.
                # Exact math guarantees u >= (a1+a2)/2 = q12/2, so the f16
                # rounding of q12 - ov can never go below ~-1 ulp of q12/2;
                # Relu(ue16 + eps) therefore stays in (0, inf) and feeds the
                # fp32-only reciprocal bit-trick safely.
                w = c["w"]
                nc.scalar.activation(c["ue32"][:, :w], c["ue16"][:, :w],
                                     Act.Relu, bias=eps_t[:, 0:1])

            def accum(c):  # ACT: loss partial rides the Ln accumulator;
                # trailing segs also get their iou partial via a Copy accum
                # (the big segs' iou partials ride the PE/PSUM reduction).
                t, w = c["t"], c["w"]
                nc.scalar.activation(
                    c["sc"][:, :w], c["iou"][:, :w], Act.Ln,
                    bias=eps_t[:, 0:1],
                    accum_out=accs[:, t : t + 1],
                )
                if t >= NBIGPS:
                    nc.scalar.activation(
                        c["sc"][:, :w], c["iou"][:, :w], Act.Copy,
                        accum_out=accs[:, NSEG + t : NSEG + t + 1],
                    )

            # Five-stage software pipeline: every cross-engine dependency
            # lands at least one full segment before its consumer, so no
            # engine queue ever blocks mid-iteration. Per-iteration queue
            # orders (front-loaded ready work first):
            #   DVE : mw,mh(k-1) | r(k-4) | dx..D(k) | wr,relus,q12(k-1)
            #   ACT : li,ic(k-5) | ue32(k-3) | abs,squares(k)
            #   Pool: ov,ue16(k-2) | hr(k-1) | iou(k-4)
            def ps_extract(bank, col, n):
                # ACT copies one PSUM bank into a small staging tile which
                # streams straight out to DRAM.
                pscp = pool.tile([1, 512], F32, tag="pscp", name=f"pscp_{col}")
                nc.scalar.copy(pscp[:, :n], bank[:, :n])
                nc.sync.dma_start(out=iouv_d[:, col : col + n],
                                  in_=pscp[:, :n])

            for k in range(NSEG + 5):
                if 4 <= k <= NSEG + 3:
                    accum(C[k - 4])
                    iou_psum(C[k - 4])
                if 2 <= k <= NSEG + 1:
                    unionst(C[k - 2])
                if k < NSEG:
                    front(k)
                if 3 <= k <= NSEG + 2:
                    recip(C[k - 3])
                    ioust_dve(C[k - 3])
                    ioust_pool(C[k - 3])
                if 1 <= k <= NSEG:
                    maxes(C[k - 1])
                    hrst(C[k - 1])
                if k < NSEG:
                    front2(C[k])
                if 2 <= k <= NSEG + 1:
                    ue32st(C[k - 2])
                if 1 <= k <= NSEG:
                    rhst(C[k - 1])
                if k == NBIGPS + 4:
                    # the PSUM group closed at iter NBIGPS+3; drain its banks
                    # while the trailing small tiles stream
                    ps_extract(psA1, 0, 512)
                    ps_extract(psA2, 512, 512)

            # accs is written only by ACT accumulators; the store needs just
            # the ACT sem tick of the final Copy, which fits the single DMA
            # wait slot.
            nc.sync.dma_start(out=acc_d[:, :], in_=accs[:, :])

    if compile_passes:
        # Bacc.compile runs generate_event_semaphores (splits multi-wait
        # instructions to satisfy the 1-wait-per-instruction HW limit),
        # extended-inst lowering, and ACT table loads.
        nc.compile()
    return nc


_NC_CACHE: list[bass.Bass] = []


def _get_nc() -> bass.Bass:
    if not _NC_CACHE:
        _NC_CACHE.append(_build())
    return _NC_CACHE[0]


def _run(inputs: dict, trace: bool = False, trace_kwargs: dict | None = None):
    outputs = np.ascontiguousarray(np.asarray(inputs["outputs"], dtype=np.float32))
    targets = np.ascontiguousarray(np.asarray(inputs["targets"], dtype=np.float32))
    assert outputs.shape == (N, 3) and targets.shape == (N, 3)

    nc = _get_nc()
    in_maps = [
        {
            "outputs": outputs[c * NS : (c + 1) * NS],
            "targets": targets[c * NS : (c + 1) * NS],
        }
        for c in range(NCORES)
    ]
    kw = {}
    if trace:
        kw["trace"] = True
        if trace_kwargs:
            kw["trace_kwargs"] = trace_kwargs
    res = run_bass_kernel_spmd(nc, in_maps, list(range(NCORES)), **kw)

    iou_sum = 0.0
    loss = 0.0
    for c in range(NCORES):
        acc = np.asarray(res.results[c]["acc"], dtype=np.float64)
        half = acc.shape[1] // 2
        loss += acc[:, :half].sum()
        iou_sum += acc[:, half:].sum()
        iou_sum += np.asarray(res.results[c]["iouv"], dtype=np.float64).sum()
    loss = -loss
    return (np.float32(loss), np.float32(iou_sum)), res


def kernel(**inputs) -> tuple:
    (loss, iou_sum), _ = _run(inputs)
    return (loss, iou_sum)



# revision 4
# speedup vs baseline: 1.1429x; 1.1429x over previous
"""IoU loss kernel for Trainium2, data-parallel over 8 NeuronCores.

Math (per box, columns = x-center, y-center, half-size s):
    w = relu(min(x+s, x'+s') - max(x-s, x'-s'))
      = relu((s+s') - max(|x-x'|, |s-s'|))          # S - max identity
    h likewise with y.
    overlap = w*h
    union   = 4s^2 + 4s'^2 - overlap = 2(S^2 + D^2) - overlap,
              S = s+s', D = s-s'
    iou     = overlap / (union + 1e-7)
    loss    = -sum(log(iou + 1e-7));  iou_sum = sum(iou)

The DMA stream (two fp32 loads per tile, 24 MiB/core total, ~70us at
the 358 GB/s per-core HBM rate) and DVE/ACT are co-bottlenecks; the
work is spread so no engine exceeds ~9.3us per KTile segment:

  DVE  (~9.3us/KTile): dx, dy, S, D (fp32 strided reads -> fp16, the
         unavoidable AoS de-interleave, ~6us), mw/mh = max(a.,aD)
         (fp16 2x), wr = S - mw, r = 1/(u+eps) via
         reciprocal_approx_fast (~18 bits).
  ACT  (~9.1us/KTile): |dx|, |dy|, |D| (Abs), relu(wr)/relu(hr),
         2S^2/2D^2 (Square with scale=sqrt2), ue32 = Relu(ue_psum)+eps
         upcast (2 half-tiles from PSUM), Ln(iou+eps) accum -> loss
         partial. All funcs live in the `natural_log` table set ->
         ONE table load total.
  Pool (~7.5us/KTile): hr = S - mh, ov = rw*rh, iou = ov*r (the three
         Q7-ucode tensor_tensor slots; mixed fp16*fp32 iou mult).
  PE   (~4.5us/KTile): ue = I*qS + I*qD + (-I)*ov accumulated in PSUM
         (identity-weight matmuls; removes the q12/ue16 elementwise
         chain from DVE/Pool), plus ones-weight matmuls accumulating
         the per-partition iou sums across all segments into two PSUM
         banks. PE drains also absorb the raw-slot recycle waits so
         the DMA stream stays wait-free.

Six-iteration software pipeline; the per-iteration engine queue orders
put ready work first so cross-engine deps are either >=1 iteration old
or produced earlier in the same iteration by an engine ahead in its
own queue. Trailing segments shrink (512/512) to cut the drain-out
tail. Host: final [128, NSEG] x 8 cores loss-partial reduction +
[1,1024] iou partials, summed in float64.
"""

import numpy as np

import concourse.bass as bass
import concourse.mybir as mybir
from concourse import tile
from concourse.bass_utils import run_bass_kernel_spmd

N = 8388608
NCORES = 8
NS = N // NCORES  # 1048576 boxes per core
P = 128
W = 1024          # boxes per partition per full tile
T = NS // (P * W)  # 8 full-tile units per core
EPS = 1e-7
RT2 = 1.4142135623730951

F32 = mybir.dt.float32
F16 = mybir.dt.float16
Op = mybir.AluOpType
Act = mybir.ActivationFunctionType


def _build(T_: int = T, W_: int = W, compile_passes: bool = True) -> bass.Bass:
    from concourse import bacc
    from concourse.tile_rust import add_dep_helper

    segs = [W_] * (T_ - 1) + [W_ // 2, W_ // 2]
    assert sum(segs) == T_ * W_
    NSEG = len(segs)
    SMALLW = W_ // 2
    H = W_ // 2  # psum bank width (512 fp32 cols)

    ns = P * W_ * T_
    nc = bacc.Bacc()
    outs_d = nc.dram_tensor("outputs", [ns, 3], F32, kind="ExternalInput")
    tars_d = nc.dram_tensor("targets", [ns, 3], F32, kind="ExternalInput")
    acc_d = nc.dram_tensor("acc", [P, NSEG], F32, kind="ExternalOutput")
    iouv_d = nc.dram_tensor("iouv", [1, 2 * H], F32, kind="ExternalOutput")

    offs = [0]
    for w in segs:
        offs.append(offs[-1] + w)

    def seg_view(dram, s):
        b0 = P * offs[s]
        return dram[b0 : b0 + P * segs[s], :].rearrange(
            "(p w) c -> p (w c)", p=P, w=segs[s]
        )

    RAWBUFS = 3

    with tile.TileContext(nc) as tc:
        with (tc.tile_pool(name="main", bufs=2) as pool,
              tc.tile_pool(name="psum", bufs=1,
                           space=bass.MemorySpace.PSUM) as psum):
            accs = pool.tile([P, NSEG], F32, tag="accs", bufs=1)
            nc.vector.memset(accs[:, :], 0.0)
            eps_t = pool.tile([P, 1], F32, tag="eps", bufs=1)
            nc.vector.memset(eps_t[:, :], EPS)
            ones = pool.tile([P, 1], F16, tag="ones", bufs=1)
            nc.vector.memset(ones[:, :], 1.0)

            # identity / -identity fp16 weights for the PSUM union
            # accumulation: iota(col - partition) is exact in fp16
            # (range +-127), is_equal against 0 marks the diagonal.
            colmp = pool.tile([P, P], F16, tag="colmp", bufs=1)
            nc.gpsimd.iota(colmp[:, :], [[1, P]], channel_multiplier=-1,
                           allow_small_or_imprecise_dtypes=True)
            ident = pool.tile([P, P], F16, tag="ident", bufs=1)
            nc.vector.tensor_scalar(ident[:, :], colmp[:, :], 0.0, None,
                                    Op.is_equal)
            nident = pool.tile([P, P], F16, tag="nident", bufs=1)
            nc.vector.tensor_scalar(nident[:, :], colmp[:, :], 0.0, -1.0,
                                    Op.is_equal, Op.mult)

            # PSUM: iou partition-sum accumulators (whole kernel), and
            # double-buffered union banks (2 half-tiles per segment).
            psA1 = psum.tile([1, H], F32, tag="psA1", bufs=1)
            psA2 = psum.tile([1, H], F32, tag="psA2", bufs=1)
            ueP1 = psum.tile([P, H], F32, tag="ueP1", bufs=2)
            ueP2 = psum.tile([P, H], F32, tag="ueP2", bufs=2)

            lastrd: list = []
            dmaO_h: list = []
            dmaT_h: list = []
            big_idx: list = []
            C: list = []

            def front(t):
                w = segs[t]
                if w <= SMALLW:
                    rawO = pool.tile([P, 3 * SMALLW], F32, tag="rawOs", bufs=3)
                    rawT = pool.tile([P, 3 * SMALLW], F32, tag="rawTs", bufs=3)
                    recycle = None
                else:
                    rawO = pool.tile([P, 3 * W_], F32, tag="rawO", bufs=RAWBUFS)
                    rawT = pool.tile([P, 3 * W_], F32, tag="rawT", bufs=RAWBUFS)
                    nbig = len(big_idx)
                    recycle = big_idx[nbig - RAWBUFS] if nbig >= RAWBUFS else None
                    big_idx.append(t)
                deps = []
                if recycle is not None:
                    dr = nc.tensor.drain(fusable=False)
                    add_dep_helper(dr.ins, lastrd[recycle].ins, sync=True,
                                   reason="absorb DVE WAR tick")
                    add_dep_helper(dr.ins, dmaO_h[recycle].ins, sync=True,
                                   reason="absorb old rawO DMA lane")
                    add_dep_helper(dr.ins, dmaT_h[recycle].ins, sync=True,
                                   reason="absorb old rawT DMA lane")
                    deps = [dr]
                dmaO = nc.sync.dma_start(out=rawO[:, : 3 * w], in_=seg_view(outs_d, t))
                dmaT = nc.sync.dma_start(out=rawT[:, : 3 * w], in_=seg_view(tars_d, t))
                for d in deps:
                    add_dep_helper(dmaO.ins, d.ins, sync=True,
                                   reason="slot guarded by PE drain")
                    add_dep_helper(dmaT.ins, d.ins, sync=True,
                                   reason="slot guarded by PE drain")
                dmaO_h.append(dmaO)
                dmaT_h.append(dmaT)

                # dx depends on BOTH input DMAs but has one sync-wait
                # slot; absorb rawT's semaphore with a tiny same-queue copy.
                dummy = pool.tile([P, 1], F32, tag="dummy")
                nc.vector.tensor_copy(dummy[:, :], rawT[:, 0:1])
                o3 = rawO[:, : 3 * w].rearrange("p (w c) -> p w c", c=3)
                t3 = rawT[:, : 3 * w].rearrange("p (w c) -> p w c", c=3)
                x1, y1, s1 = o3[:, :, 0], o3[:, :, 1], o3[:, :, 2]
                x2, y2, s2 = t3[:, :, 0], t3[:, :, 1], t3[:, :, 2]

                c = {"t": t, "w": w}
                for nm, nb in (("dx", 2), ("dy", 2), ("S", 3), ("D", 2),
                               ("adx", 2), ("ady", 2), ("aD", 2),
                               ("mw", 2), ("mh", 2), ("wr", 2), ("hr", 2),
                               ("rw", 2), ("rh", 2), ("qS", 3), ("qD", 3),
                               ("ov", 2), ("iou", 2), ("sc", 1)):
                    c[nm] = pool.tile([P, W_], F16, tag=nm, bufs=nb,
                                      name=f"{nm}_{t}")
                for nm in ("ue32", "r"):
                    c[nm] = pool.tile([P, W_], F32, tag=nm, name=f"{nm}_{t}")

                nc.vector.tensor_tensor(c["dx"][:, :w], x1, x2, Op.subtract)
                c["ins"] = (x1, y1, s1, x2, y2, s2)
                C.append(c)

            def front2(c):
                x1, y1, s1, x2, y2, s2 = c["ins"]
                w = c["w"]
                nc.vector.tensor_tensor(c["dy"][:, :w], y1, y2, Op.subtract)
                nc.vector.tensor_tensor(c["S"][:, :w], s1, s2, Op.add)
                lastrd.append(
                    nc.vector.tensor_tensor(c["D"][:, :w], s1, s2, Op.subtract))

            def absd(c):  # ACT: |dx|, |dy|, |D| and the scaled squares
                w = c["w"]
                nc.scalar.activation(c["adx"][:, :w], c["dx"][:, :w], Act.Abs)
                nc.scalar.activation(c["ady"][:, :w], c["dy"][:, :w], Act.Abs)
                nc.scalar.activation(c["aD"][:, :w], c["D"][:, :w], Act.Abs)
                nc.scalar.activation(c["qS"][:, :w], c["S"][:, :w], Act.Square,
                                     scale=RT2)
                nc.scalar.activation(c["qD"][:, :w], c["D"][:, :w], Act.Square,
                                     scale=RT2)

            def mids(c):  # DVE: thresholds (fp16 2x), w-extent precursor
                w = c["w"]
                nc.vector.tensor_tensor(c["mw"][:, :w], c["adx"][:, :w],
                                        c["aD"][:, :w], Op.max)
                nc.vector.tensor_tensor(c["mh"][:, :w], c["ady"][:, :w],
                                        c["aD"][:, :w], Op.max)
                nc.vector.tensor_sub(c["wr"][:, :w], c["S"][:, :w], c["mw"][:, :w])

            def hrst(c):  # Pool: hr = S - mh
                w = c["w"]
                nc.gpsimd.tensor_tensor(c["hr"][:, :w], c["S"][:, :w],
                                        c["mh"][:, :w], Op.subtract)

            def relus(c):  # ACT: clamp both extents
                w = c["w"]
                nc.scalar.activation(c["rw"][:, :w], c["wr"][:, :w], Act.Relu)
                nc.scalar.activation(c["rh"][:, :w], c["hr"][:, :w], Act.Relu)

            def ovst(c):  # Pool: ov = rw*rh
                w = c["w"]
                nc.gpsimd.tensor_tensor(c["ov"][:, :w], c["rw"][:, :w],
                                        c["rh"][:, :w], Op.mult)

            def uemm(c):  # PE: ue = 2S^2 + 2D^2 - ov accumulated in PSUM
                w = c["w"]
                banks = [(ueP1, 0)] if w <= H else [(ueP1, 0), (ueP2, H)]
                c["banks"] = banks
                for bank, o in banks:
                    nc.tensor.matmul(bank[:, :], ident[:, :],
                                     c["qS"][:, o : o + H], start=True, stop=False)
                    nc.tensor.matmul(bank[:, :], ident[:, :],
                                     c["qD"][:, o : o + H], start=False, stop=False)
                    nc.tensor.matmul(bank[:, :], nident[:, :],
                                     c["ov"][:, o : o + H], start=False, stop=True)

            def ue32st(c):  # ACT: fp32 upcast of PSUM union with eps floor
                for bank, o in c["banks"]:
                    nc.scalar.activation(c["ue32"][:, o : o + H], bank[:, :],
                                         Act.Relu, bias=eps_t[:, 0:1])

            def recip(c):  # DVE: r = 1/(u+eps), ~18 bits
                w = c["w"]
                nc.vector.reciprocal_approx_fast(c["r"][:, :w], c["ue32"][:, :w])

            def ioust(c):  # Pool: iou = ov * r (fp16 * fp32 -> fp16)
                w = c["w"]
                nc.gpsimd.tensor_tensor(c["iou"][:, :w], c["ov"][:, :w],
                                        c["r"][:, :w], Op.mult)

            def iou_psum(c):  # PE: per-seg partition-sums into PSUM banks
                t, w = c["t"], c["w"]
                nc.tensor.matmul(psA1[:, :], ones[:, :], c["iou"][:, :H],
                                 start=(t == 0), stop=(t == NSEG - 1))
                if w > H:
                    nc.tensor.matmul(psA2[:, :], ones[:, :], c["iou"][:, H : 2 * H],
                                     start=(t == 0), stop=(t == T_ - 2))

            def accum(c):  # ACT: loss partial rides the Ln accumulator
                t, w = c["t"], c["w"]
                nc.scalar.activation(
                    c["sc"][:, :w], c["iou"][:, :w], Act.Ln,
                    bias=eps_t[:, 0:1],
                    accum_out=accs[:, t : t + 1],
                )

            def ps_extract(bank, col, n):
                pscp = pool.tile([1, H], F32, tag="pscp", name=f"pscp_{col}")
                nc.scalar.copy(pscp[:, :n], bank[:, :n])
                nc.sync.dma_start(out=iouv_d[:, col : col + n],
                                  in_=pscp[:, :n])

            # Six-stage pipeline; per-iteration engine queue orders:
            #   ACT : ln(k-6) | rw,rh(k-3) | abs,squares(k-1) | ue32(k-3)
            #   Pool: iou(k-5) | ov(k-3) | hr(k-2)
            #   PE  : iou_psum(k-6) | uemm(k-3) | recycle drain(k)
            #   DVE : mw,mh,wr(k-2) | recip(k-4) | dummy,dx(k) | dy,S,D(k)
            for k in range(NSEG + 7):
                if 6 <= k <= NSEG + 5:
                    accum(C[k - 6])
                    iou_psum(C[k - 6])
                if 5 <= k <= NSEG + 4:
                    ioust(C[k - 5])
                if 3 <= k <= NSEG + 2:
                    relus(C[k - 3])
                    ovst(C[k - 3])
                    uemm(C[k - 3])
                if 2 <= k <= NSEG + 1:
                    mids(C[k - 2])
                if 4 <= k <= NSEG + 3:
                    recip(C[k - 4])
                if k < NSEG:
                    front(k)
                if 2 <= k <= NSEG + 1:
                    hrst(C[k - 2])
                if 1 <= k <= NSEG:
                    absd(C[k - 1])
                if k < NSEG:
                    front2(C[k])
                if 3 <= k <= NSEG + 2:
                    ue32st(C[k - 3])
                if k == T_ + 5:  # psA2 closed at iter T_+4 (seg T_-2)
                    ps_extract(psA2, H, H)

            ps_extract(psA1, 0, H)
            nc.sync.dma_start(out=acc_d[:, :], in_=accs[:, :])

    if compile_passes:
        nc.compile()
    return nc


_NC_CACHE: list[bass.Bass] = []


def _get_nc() -> bass.Bass:
    if not _NC_CACHE:
        _NC_CACHE.append(_build())
    return _NC_CACHE[0]


def _run(inputs: dict, trace: bool = False, trace_kwargs: dict | None = None):
    outputs = np.ascontiguousarray(np.asarray(inputs["outputs"], dtype=np.float32))
    targets = np.ascontiguousarray(np.asarray(inputs["targets"], dtype=np.float32))
    assert outputs.shape == (N, 3) and targets.shape == (N, 3)

    nc = _get_nc()
    in_maps = [
        {
            "outputs": outputs[c * NS : (c + 1) * NS],
            "targets": targets[c * NS : (c + 1) * NS],
        }
        for c in range(NCORES)
    ]
    kw = {}
    if trace:
        kw["trace"] = True
        if trace_kwargs:
            kw["trace_kwargs"] = trace_kwargs
    res = run_bass_kernel_spmd(nc, in_maps, list(range(NCORES)), **kw)

    iou_sum = 0.0
    loss = 0.0
    for c in range(NCORES):
        acc = np.asarray(res.results[c]["acc"], dtype=np.float64)
        loss += acc.sum()
        iou_sum += np.asarray(res.results[c]["iouv"], dtype=np.float64).sum()
    loss = -loss
    return (np.float32(loss), np.float32(iou_sum)), res


def kernel(**inputs) -> tuple:
    (loss, iou_sum), _ = _run(inputs)
    return (loss, iou_sum)


# revision 6
# speedup vs baseline: 1.1997x; 1.0497x over previous
"""IoU loss kernel for Trainium2, data-parallel over 8 NeuronCores.

Math (per box, columns = x-center, y-center, half-size s):
    w = relu(min(x+s, x'+s') - max(x-s, x'-s'))
      = relu((s+s') - max(|x-x'|, |s-s'|))          # S - max identity
    h likewise with y.
    overlap = w*h
    union   = 4s^2 + 4s'^2 - overlap = 2(S^2 + D^2) - overlap,
              S = s+s', D = s-s'
    iou     = overlap / (union + 1e-7)
    loss    = -sum(log(iou + 1e-7));  iou_sum = sum(iou)

The DMA stream (two fp32 loads per tile, 24 MiB/core total, ~70us at
the 358 GB/s per-core HBM rate) and DVE/ACT are co-bottlenecks; the
work is spread so no engine exceeds ~8.5us per KTile segment. The key
SBUF-bandwidth trick: four strided fp32 reads of the AoS raw data
would fetch the same 32B beats four times (~98KB/partition/seg);
instead ONE contiguous fp32 subtract m3 = rawO - rawT covers dx, dy
and D at full beat utilization, and the column extraction rides the
stride-blind ACT ops. Only S = s1+s2 stays strided.

  DVE  (~8.4us/KTile): m3 (contiguous fp32 -> fp16, 3W cols), S
         (strided), mw/mh = max(a.,aD) (fp16 2x), wr = S - mw,
         relu(wr) (tensor_scalar 4x), r = 1/(u+eps) via
         reciprocal_approx_fast (~18 bits).
  ACT  (~8.2us/KTile): |dx|, |dy|, |D| (Abs on m3's strided columns),
         relu(hr), 2S^2/2D^2 (Square with scale=sqrt2), ue32 =
         Relu(ue_psum)+eps upcast (2 half-tiles from PSUM),
         Ln(iou+eps) accum -> loss partial. All funcs live in the
         `natural_log` table set -> ONE table load total.
  Pool (~7.5us/KTile): hr = S - mh, ov = rw*rh, iou = ov*r (the three
         Q7-ucode tensor_tensor slots; mixed fp16*fp32 iou mult).
  PE   (~4.5us/KTile): ue = I*qS + I*qD + (-I)*ov accumulated in PSUM
         (identity-weight matmuls; removes the q12/ue16 elementwise
         chain from DVE/Pool), plus ones-weight matmuls accumulating
         the per-partition iou sums across all segments into two PSUM
         banks. PE drains also absorb the raw-slot recycle waits so
         the DMA stream stays wait-free.

Six-iteration software pipeline; the per-iteration engine queue orders
put ready work first so cross-engine deps are either >=1 iteration old
or produced earlier in the same iteration by an engine ahead in its
own queue. Trailing segments shrink (512/512) to cut the drain-out
tail. Host: final [128, NSEG] x 8 cores loss-partial reduction +
[1,1024] iou partials, summed in float64.
"""

import numpy as np

import concourse.bass as bass
import concourse.mybir as mybir
from concourse import tile
from concourse.bass_utils import run_bass_kernel_spmd

N = 8388608
NCORES = 8
NS = N // NCORES  # 1048576 boxes per core
P = 128
W = 1024          # boxes per partition per full tile
T = NS // (P * W)  # 8 full-tile units per core
EPS = 1e-7
RT2 = 1.4142135623730951

F32 = mybir.dt.float32
F16 = mybir.dt.float16
Op = mybir.AluOpType
Act = mybir.ActivationFunctionType


def _build(T_: int = T, W_: int = W, compile_passes: bool = True) -> bass.Bass:
    from concourse import bacc
    from concourse.tile_rust import add_dep_helper

    segs = [W_] * (T_ - 1) + [W_ // 2, W_ // 2]
    assert sum(segs) == T_ * W_
    NSEG = len(segs)
    SMALLW = W_ // 2
    H = W_ // 2  # psum bank width (512 fp32 cols)

    ns = P * W_ * T_
    nc = bacc.Bacc()
    outs_d = nc.dram_tensor("outputs", [ns, 3], F32, kind="ExternalInput")
    tars_d = nc.dram_tensor("targets", [ns, 3], F32, kind="ExternalInput")
    acc_d = nc.dram_tensor("acc", [P, NSEG], F32, kind="ExternalOutput")
    iouv_d = nc.dram_tensor("iouv", [1, 2 * H], F32, kind="ExternalOutput")

    offs = [0]
    for w in segs:
        offs.append(offs[-1] + w)

    def seg_view(dram, s):
        b0 = P * offs[s]
        return dram[b0 : b0 + P * segs[s], :].rearrange(
            "(p w) c -> p (w c)", p=P, w=segs[s]
        )

    RAWBUFS = 3

    with tile.TileContext(nc) as tc:
        with (tc.tile_pool(name="main", bufs=2) as pool,
              tc.tile_pool(name="psum", bufs=1,
                           space=bass.MemorySpace.PSUM) as psum):
            accs = pool.tile([P, NSEG], F32, tag="accs", bufs=1)
            nc.vector.memset(accs[:, :], 0.0)
            eps_t = pool.tile([P, 1], F32, tag="eps", bufs=1)
            nc.vector.memset(eps_t[:, :], EPS)
            ones = pool.tile([P, 1], F16, tag="ones", bufs=1)
            nc.vector.memset(ones[:, :], 1.0)

            # identity / -identity fp16 weights for the PSUM union
            # accumulation: iota(col - partition) is exact in fp16
            # (range +-127), is_equal against 0 marks the diagonal.
            colmp = pool.tile([P, P], F16, tag="colmp", bufs=1)
            nc.gpsimd.iota(colmp[:, :], [[1, P]], channel_multiplier=-1,
                           allow_small_or_imprecise_dtypes=True)
            ident = pool.tile([P, P], F16, tag="ident", bufs=1)
            nc.vector.tensor_scalar(ident[:, :], colmp[:, :], 0.0, None,
                                    Op.is_equal)
            nident = pool.tile([P, P], F16, tag="nident", bufs=1)
            nc.vector.tensor_scalar(nident[:, :], colmp[:, :], 0.0, -1.0,
                                    Op.is_equal, Op.mult)

            # PSUM: iou partition-sum accumulators (whole kernel), and
            # double-buffered union banks (2 half-tiles per segment).
            psA1 = psum.tile([1, H], F32, tag="psA1", bufs=1)
            psA2 = psum.tile([1, H], F32, tag="psA2", bufs=1)
            ueP1 = psum.tile([P, H], F32, tag="ueP1", bufs=2)
            ueP2 = psum.tile([P, H], F32, tag="ueP2", bufs=2)

            lastrd: list = []
            dmaO_h: list = []
            dmaT_h: list = []
            big_idx: list = []
            C: list = []

            def front(t):
                w = segs[t]
                if w <= SMALLW:
                    rawO = pool.tile([P, 3 * SMALLW], F32, tag="rawOs", bufs=3)
                    rawT = pool.tile([P, 3 * SMALLW], F32, tag="rawTs", bufs=3)
                    recycle = None
                else:
                    rawO = pool.tile([P, 3 * W_], F32, tag="rawO", bufs=RAWBUFS)
                    rawT = pool.tile([P, 3 * W_], F32, tag="rawT", bufs=RAWBUFS)
                    nbig = len(big_idx)
                    recycle = big_idx[nbig - RAWBUFS] if nbig >= RAWBUFS else None
                    big_idx.append(t)
                deps = []
                if recycle is not None:
                    dr = nc.tensor.drain(fusable=False)
                    add_dep_helper(dr.ins, lastrd[recycle].ins, sync=True,
                                   reason="absorb DVE WAR tick")
                    add_dep_helper(dr.ins, dmaO_h[recycle].ins, sync=True,
                                   reason="absorb old rawO DMA lane")
                    add_dep_helper(dr.ins, dmaT_h[recycle].ins, sync=True,
                                   reason="absorb old rawT DMA lane")
                    deps = [dr]
                dmaO = nc.sync.dma_start(out=rawO[:, : 3 * w], in_=seg_view(outs_d, t))
                dmaT = nc.sync.dma_start(out=rawT[:, : 3 * w], in_=seg_view(tars_d, t))
                for d in deps:
                    add_dep_helper(dmaO.ins, d.ins, sync=True,
                                   reason="slot guarded by PE drain")
                    add_dep_helper(dmaT.ins, d.ins, sync=True,
                                   reason="slot guarded by PE drain")
                dmaO_h.append(dmaO)
                dmaT_h.append(dmaT)

                # m3 depends on BOTH input DMAs but has one sync-wait
                # slot; absorb rawT's semaphore with a tiny same-queue copy.
                dummy = pool.tile([P, 1], F32, tag="dummy")
                nc.vector.tensor_copy(dummy[:, :], rawT[:, 0:1])
                o3 = rawO[:, : 3 * w].rearrange("p (w c) -> p w c", c=3)
                t3 = rawT[:, : 3 * w].rearrange("p (w c) -> p w c", c=3)
                s1, s2 = o3[:, :, 2], t3[:, :, 2]

                c = {"t": t, "w": w}
                c["m3"] = pool.tile([P, 3 * W_], F16, tag="m3", bufs=2,
                                    name=f"m3_{t}")
                for nm, nb in (("S", 3),
                               ("adx", 2), ("ady", 2), ("aD", 2),
                               ("mw", 2), ("mh", 2), ("wr", 2), ("hr", 2),
                               ("rw", 2), ("rh", 2), ("qS", 3), ("qD", 3),
                               ("ov", 2), ("iou", 2), ("sc", 1)):
                    c[nm] = pool.tile([P, W_], F16, tag=nm, bufs=nb,
                                      name=f"{nm}_{t}")
                for nm in ("ue32", "r"):
                    c[nm] = pool.tile([P, W_], F32, tag=nm, name=f"{nm}_{t}")

                # the whole AoS de-interleave collapses into ONE contiguous
                # fp32 subtract (full 32B-beat utilization); the strided
                # column extraction happens on ACT, which is stride-blind.
                nc.vector.tensor_tensor(c["m3"][:, : 3 * w],
                                        rawO[:, : 3 * w], rawT[:, : 3 * w],
                                        Op.subtract)
                c["ins"] = (s1, s2)
                C.append(c)

            def front2(c):
                s1, s2 = c["ins"]
                w = c["w"]
                lastrd.append(
                    nc.vector.tensor_tensor(c["S"][:, :w], s1, s2, Op.add))

            def absd(c):  # ACT: |dx|, |dy|, |D| and the scaled squares,
                # reading the interleaved m3 columns (stride-blind engine)
                w = c["w"]
                d3 = c["m3"][:, : 3 * w].rearrange("p (w c) -> p w c", c=3)
                dx, dy, D = d3[:, :, 0], d3[:, :, 1], d3[:, :, 2]
                nc.scalar.activation(c["adx"][:, :w], dx, Act.Abs)
                nc.scalar.activation(c["ady"][:, :w], dy, Act.Abs)
                nc.scalar.activation(c["aD"][:, :w], D, Act.Abs)
                nc.scalar.activation(c["qS"][:, :w], c["S"][:, :w], Act.Square,
                                     scale=RT2)
                nc.scalar.activation(c["qD"][:, :w], D, Act.Square,
                                     scale=RT2)

            def mids(c):  # DVE: thresholds (fp16 2x), w-extent + its relu
                w = c["w"]
                nc.vector.tensor_tensor(c["mw"][:, :w], c["adx"][:, :w],
                                        c["aD"][:, :w], Op.max)
                nc.vector.tensor_tensor(c["mh"][:, :w], c["ady"][:, :w],
                                        c["aD"][:, :w], Op.max)
                nc.vector.tensor_sub(c["wr"][:, :w], c["S"][:, :w], c["mw"][:, :w])
                nc.vector.tensor_scalar_max(c["rw"][:, :w], c["wr"][:, :w], 0.0)

            def hrst(c):  # Pool: hr = S - mh
                w = c["w"]
                nc.gpsimd.tensor_tensor(c["hr"][:, :w], c["S"][:, :w],
                                        c["mh"][:, :w], Op.subtract)

            def relus(c):  # ACT: clamp the h-extent
                w = c["w"]
                nc.scalar.activation(c["rh"][:, :w], c["hr"][:, :w], Act.Relu)

            def ovst(c):  # Pool: ov = rw*rh
                w = c["w"]
                nc.gpsimd.tensor_tensor(c["ov"][:, :w], c["rw"][:, :w],
                                        c["rh"][:, :w], Op.mult)

            def uemm(c):  # PE: ue = 2S^2 + 2D^2 - ov accumulated in PSUM
                w = c["w"]
                banks = [(ueP1, 0)] if w <= H else [(ueP1, 0), (ueP2, H)]
                c["banks"] = banks
                for bank, o in banks:
                    nc.tensor.matmul(bank[:, :], ident[:, :],
                                     c["qS"][:, o : o + H], start=True, stop=False)
                    nc.tensor.matmul(bank[:, :], ident[:, :],
                                     c["qD"][:, o : o + H], start=False, stop=False)
                    nc.tensor.matmul(bank[:, :], nident[:, :],
                                     c["ov"][:, o : o + H], start=False, stop=True)

            def ue32st(c):  # ACT: fp32 upcast of PSUM union with eps floor
                for bank, o in c["banks"]:
                    nc.scalar.activation(c["ue32"][:, o : o + H], bank[:, :],
                                         Act.Relu, bias=eps_t[:, 0:1])

            def recip(c):  # DVE: r = 1/(u+eps), ~18 bits
                w = c["w"]
                nc.vector.reciprocal_approx_fast(c["r"][:, :w], c["ue32"][:, :w])

            def ioust(c):  # Pool: iou = ov * r (fp16 * fp32 -> fp16)
                w = c["w"]
                nc.gpsimd.tensor_tensor(c["iou"][:, :w], c["ov"][:, :w],
                                        c["r"][:, :w], Op.mult)

            def iou_psum(c):  # PE: per-seg partition-sums into PSUM banks
                t, w = c["t"], c["w"]
                nc.tensor.matmul(psA1[:, :], ones[:, :], c["iou"][:, :H],
                                 start=(t == 0), stop=(t == NSEG - 1))
                if w > H:
                    nc.tensor.matmul(psA2[:, :], ones[:, :], c["iou"][:, H : 2 * H],
                                     start=(t == 0), stop=(t == T_ - 2))

            def accum(c):  # ACT: loss partial rides the Ln accumulator
                t, w = c["t"], c["w"]
                nc.scalar.activation(
                    c["sc"][:, :w], c["iou"][:, :w], Act.Ln,
                    bias=eps_t[:, 0:1],
                    accum_out=accs[:, t : t + 1],
                )

            def ps_extract(bank, col, n):
                pscp = pool.tile([1, H], F32, tag="pscp", name=f"pscp_{col}")
                nc.scalar.copy(pscp[:, :n], bank[:, :n])
                nc.sync.dma_start(out=iouv_d[:, col : col + n],
                                  in_=pscp[:, :n])

            # Six-stage pipeline; per-iteration engine queue orders:
            #   ACT : ln(k-6) | rw,rh(k-3) | abs,squares(k-1) | ue32(k-3)
            #   Pool: iou(k-5) | ov(k-3) | hr(k-2)
            #   PE  : iou_psum(k-6) | uemm(k-3) | recycle drain(k)
            #   DVE : mw,mh,wr(k-2) | recip(k-4) | dummy,dx(k) | dy,S,D(k)
            for k in range(NSEG + 7):
                if 6 <= k <= NSEG + 5:
                    accum(C[k - 6])
                    iou_psum(C[k - 6])
                if 5 <= k <= NSEG + 4:
                    ioust(C[k - 5])
                if 3 <= k <= NSEG + 2:
                    relus(C[k - 3])
                    ovst(C[k - 3])
                    uemm(C[k - 3])
                if 2 <= k <= NSEG + 1:
                    mids(C[k - 2])
                if 4 <= k <= NSEG + 3:
                    recip(C[k - 4])
                if k < NSEG:
                    front(k)
                if 2 <= k <= NSEG + 1:
                    hrst(C[k - 2])
                if 1 <= k <= NSEG:
                    absd(C[k - 1])
                if k < NSEG:
                    front2(C[k])
                if 3 <= k <= NSEG + 2:
                    ue32st(C[k - 3])
                if k == T_ + 5:  # psA2 closed at iter T_+4 (seg T_-2)
                    ps_extract(psA2, H, H)

            ps_extract(psA1, 0, H)
            nc.sync.dma_start(out=acc_d[:, :], in_=accs[:, :])

    if compile_passes:
        nc.compile()
    return nc


_NC_CACHE: list[bass.Bass] = []


def _get_nc() -> bass.Bass:
    if not _NC_CACHE:
        _NC_CACHE.append(_build())
    return _NC_CACHE[0]


def _run(inputs: dict, trace: bool = False, trace_kwargs: dict | None = None):
    outputs = np.ascontiguousarray(np.asarray(inputs["outputs"], dtype=np.float32))
    targets = np.ascontiguousarray(np.asarray(inputs["targets"], dtype=np.float32))
    assert outputs.shape == (N, 3) and targets.shape == (N, 3)

    nc = _get_nc()
    in_maps = [
        {
            "outputs": outputs[c * NS : (c + 1) * NS],
            "targets": targets[c * NS : (c + 1) * NS],
        }
        for c in range(NCORES)
    ]
    kw = {}
    if trace:
        kw["trace"] = True
        if trace_kwargs:
            kw["trace_kwargs"] = trace_kwargs
    res = run_bass_kernel_spmd(nc, in_maps, list(range(NCORES)), **kw)

    iou_sum = 0.0
    loss = 0.0
    for c in range(NCORES):
        acc = np.asarray(res.results[c]["acc"], dtype=np.float64)
        loss += acc.sum()
        iou_sum += np.asarray(res.results[c]["iouv"], dtype=np.float64).sum()
    loss = -loss
    return (np.float32(loss), np.float32(iou_sum)), res


def kernel(**inputs) -> tuple:
    (loss, iou_sum), _ = _run(inputs)
    return (loss, iou_sum)


# revision 19
# speedup vs baseline: 1.2091x; 1.0079x over previous
"""IoU loss kernel for Trainium2, data-parallel over 8 NeuronCores.

Math (per box, columns = x-center, y-center, half-size s):
    w = relu(min(x+s, x'+s') - max(x-s, x'-s'))
      = relu((s+s') - max(|x-x'|, |s-s'|))          # S - max identity
    h likewise with y.
    overlap = w*h
    union   = 4s^2 + 4s'^2 - overlap = 2(S^2 + D^2) - overlap,
              S = s+s', D = s-s'
    iou     = overlap / (union + 1e-7)
    loss    = -sum(log(iou + 1e-7));  iou_sum = sum(iou)

The DMA stream (two fp32 loads per tile, 24 MiB/core total, ~70us at
the 358 GB/s per-core HBM rate) and DVE/ACT are co-bottlenecks; the
work is spread so no engine exceeds ~8.5us per KTile segment. The key
SBUF-bandwidth trick: four strided fp32 reads of the AoS raw data
would fetch the same 32B beats four times (~98KB/partition/seg);
instead ONE contiguous fp32 subtract m3 = rawO - rawT covers dx, dy
and D at full beat utilization, and the column extraction rides the
stride-blind ACT ops. Only S = s1+s2 stays strided.

  DVE  (~8.4us/KTile): m3 (contiguous fp32 -> fp16, 3W cols), S
         (strided), mw/mh = max(a.,aD) (fp16 2x), wr = S - mw,
         relu(wr) (tensor_scalar 4x), r = 1/(u+eps) via
         reciprocal_approx_fast (~18 bits).
  ACT  (~8.2us/KTile): |dx|, |dy|, |D| (Abs on m3's strided columns),
         relu(hr), 2S^2/2D^2 (Square with scale=sqrt2), ue32 =
         Relu(ue_psum)+eps upcast (2 half-tiles from PSUM),
         Ln(iou+eps) accum -> loss partial. All funcs live in the
         `natural_log` table set -> ONE table load total.
  Pool (~7.5us/KTile): hr = S - mh, ov = rw*rh, iou = ov*r (the three
         Q7-ucode tensor_tensor slots; mixed fp16*fp32 iou mult).
  PE   (~4.5us/KTile): ue = I*qS + I*qD + (-I)*ov accumulated in PSUM
         (identity-weight matmuls; removes the q12/ue16 elementwise
         chain from DVE/Pool), plus ones-weight matmuls accumulating
         the per-partition iou sums across all segments into two PSUM
         banks. PE drains also absorb the raw-slot recycle waits so
         the DMA stream stays wait-free.

Six-iteration software pipeline; the per-iteration engine queue orders
put ready work first so cross-engine deps are either >=1 iteration old
or produced earlier in the same iteration by an engine ahead in its
own queue. Trailing segments shrink (512/512) to cut the drain-out
tail. Host: final [128, NSEG] x 8 cores loss-partial reduction +
[1,1024] iou partials, summed in float64.
"""

import numpy as np

import concourse.bass as bass
import concourse.mybir as mybir
from concourse import tile
from concourse.bass_utils import run_bass_kernel_spmd

N = 8388608
NCORES = 8
NS = N // NCORES  # 1048576 boxes per core
P = 128
W = 1024          # boxes per partition per full tile
T = NS // (P * W)  # 8 full-tile units per core
EPS = 1e-7
RT2 = 1.4142135623730951

F32 = mybir.dt.float32
F16 = mybir.dt.float16
Op = mybir.AluOpType
Act = mybir.ActivationFunctionType


def _build(T_: int = T, W_: int = W, compile_passes: bool = True) -> bass.Bass:
    from concourse import bacc
    from concourse.tile_rust import add_dep_helper

    # small leading segment cuts the pipeline-fill stall (DVE waits on
    # the first DMA); small trailing segment cuts the drain-out tail.
    segs = [W_ // 2] + [W_] * (T_ - 1) + [W_ // 2]
    assert sum(segs) == T_ * W_
    NSEG = len(segs)
    SMALLW = W_ // 2
    H = W_ // 2  # psum bank width (512 fp32 cols)
    BIG_LO, BIG_HI = 1, T_ - 1  # segs with width W_ (inclusive range)

    ns = P * W_ * T_
    nc = bacc.Bacc()
    outs_d = nc.dram_tensor("outputs", [ns, 3], F32, kind="ExternalInput")
    tars_d = nc.dram_tensor("targets", [ns, 3], F32, kind="ExternalInput")
    acc_d = nc.dram_tensor("acc", [P, NSEG], F32, kind="ExternalOutput")
    iouv_d = nc.dram_tensor("iouv", [1, 2 * H], F32, kind="ExternalOutput")

    offs = [0]
    for w in segs:
        offs.append(offs[-1] + w)

    def seg_view(dram, s):
        b0 = P * offs[s]
        return dram[b0 : b0 + P * segs[s], :].rearrange(
            "(p w) c -> p (w c)", p=P, w=segs[s]
        )

    RAWBUFS = 3

    with tile.TileContext(nc) as tc:
        with (tc.tile_pool(name="main", bufs=2) as pool,
              tc.tile_pool(name="psum", bufs=1,
                           space=bass.MemorySpace.PSUM) as psum):
            accs = pool.tile([P, NSEG], F32, tag="accs", bufs=1)
            nc.vector.memset(accs[:, :], 0.0)
            eps_t = pool.tile([P, 1], F32, tag="eps", bufs=1)
            nc.vector.memset(eps_t[:, :], EPS)
            ones = pool.tile([P, 1], F16, tag="ones", bufs=1)
            nc.vector.memset(ones[:, :], 1.0)

            # identity / -identity fp16 weights for the PSUM union
            # accumulation: iota(col - partition) is exact in fp16
            # (range +-127), is_equal against 0 marks the diagonal.
            colmp = pool.tile([P, P], F16, tag="colmp", bufs=1)
            nc.gpsimd.iota(colmp[:, :], [[1, P]], channel_multiplier=-1,
                           allow_small_or_imprecise_dtypes=True)
            ident = pool.tile([P, P], F16, tag="ident", bufs=1)
            nc.vector.tensor_scalar(ident[:, :], colmp[:, :], 0.0, None,
                                    Op.is_equal)
            nident = pool.tile([P, P], F16, tag="nident", bufs=1)
            nc.vector.tensor_scalar(nident[:, :], colmp[:, :], 0.0, -1.0,
                                    Op.is_equal, Op.mult)

            # eps enters the union inside PSUM: a bf16 rank-full matmul
            # epsw.T @ ones2 adds 128 * (eps/128) = eps to every element,
            # so the reciprocal can read PSUM directly (no ACT upcast op).
            BF16 = mybir.dt.bfloat16
            epsw = pool.tile([P, P], BF16, tag="epsw", bufs=1)
            nc.vector.memset(epsw[:, :], EPS / P)
            ones2 = pool.tile([P, H], BF16, tag="ones2", bufs=1)
            nc.vector.memset(ones2[:, :], 1.0)

            # PSUM: iou partition-sum accumulators (whole kernel) plus the
            # union banks. The union banks are single-buffered and shared
            # across segments: issuing recip(s) BEFORE uemm(s+1) makes the
            # framework serialize the next segment's matmuls behind the
            # previous reciprocal read (WAR) — PE has slack to absorb it.
            psA1 = psum.tile([1, H], F32, tag="psA1", bufs=1)
            psA2 = psum.tile([1, H], F32, tag="psA2", bufs=1)
            ueP1 = psum.tile([P, H], F32, tag="ueP1", bufs=1)
            ueP2 = psum.tile([P, H], F32, tag="ueP2", bufs=1)

            lastrd: list = []
            dmaO_h: list = []
            dmaT_h: list = []
            big_idx: list = []
            C: list = []

            def front(t):
                w = segs[t]
                if w <= SMALLW:
                    rawO = pool.tile([P, 3 * SMALLW], F32, tag="rawOs", bufs=3)
                    rawT = pool.tile([P, 3 * SMALLW], F32, tag="rawTs", bufs=3)
                    recycle = None
                else:
                    rawO = pool.tile([P, 3 * W_], F32, tag="rawO", bufs=RAWBUFS)
                    rawT = pool.tile([P, 3 * W_], F32, tag="rawT", bufs=RAWBUFS)
                    nbig = len(big_idx)
                    recycle = big_idx[nbig - RAWBUFS] if nbig >= RAWBUFS else None
                    big_idx.append(t)
                deps = []
                if recycle is not None:
                    dr = nc.tensor.drain(fusable=False)
                    add_dep_helper(dr.ins, lastrd[recycle].ins, sync=True,
                                   reason="absorb DVE WAR tick")
                    add_dep_helper(dr.ins, dmaO_h[recycle].ins, sync=True,
                                   reason="absorb old rawO DMA lane")
                    add_dep_helper(dr.ins, dmaT_h[recycle].ins, sync=True,
                                   reason="absorb old rawT DMA lane")
                    deps = [dr]
                dmaO = nc.sync.dma_start(out=rawO[:, : 3 * w], in_=seg_view(outs_d, t))
                dmaT = nc.sync.dma_start(out=rawT[:, : 3 * w], in_=seg_view(tars_d, t))
                for d in deps:
                    add_dep_helper(dmaO.ins, d.ins, sync=True,
                                   reason="slot guarded by PE drain")
                    add_dep_helper(dmaT.ins, d.ins, sync=True,
                                   reason="slot guarded by PE drain")
                dmaO_h.append(dmaO)
                dmaT_h.append(dmaT)

                # m3 depends on BOTH input DMAs but has one sync-wait
                # slot; absorb rawT's semaphore with a tiny same-queue copy.
                dummy = pool.tile([P, 1], F32, tag="dummy")
                nc.vector.tensor_copy(dummy[:, :], rawT[:, 0:1])
                o3 = rawO[:, : 3 * w].rearrange("p (w c) -> p w c", c=3)
                t3 = rawT[:, : 3 * w].rearrange("p (w c) -> p w c", c=3)
                s1, s2 = o3[:, :, 2], t3[:, :, 2]

                c = {"t": t, "w": w}
                c["m3"] = pool.tile([P, 3 * W_], F16, tag="m3", bufs=2,
                                    name=f"m3_{t}")
                for nm, nb in (("S", 3),
                               ("adx", 2), ("ady", 2), ("aD", 2),
                               ("mw", 2), ("mh", 2), ("wr", 2), ("hr", 2),
                               ("rh", 2), ("qS", 3), ("qD", 3),
                               ("ov", 2), ("iou", 2), ("sc", 1)):
                    c[nm] = pool.tile([P, W_], F16, tag=nm, bufs=nb,
                                      name=f"{nm}_{t}")
                c["r"] = pool.tile([P, W_], F32, tag="r", name=f"r_{t}")
                c["banks"] = ([(ueP1, 0)] if w <= H
                              else [(ueP1, 0), (ueP2, H)])

                # the whole AoS de-interleave collapses into ONE contiguous
                # fp32 subtract (full 32B-beat utilization); the strided
                # column extraction happens on ACT, which is stride-blind.
                nc.vector.tensor_tensor(c["m3"][:, : 3 * w],
                                        rawO[:, : 3 * w], rawT[:, : 3 * w],
                                        Op.subtract)
                c["ins"] = (s1, s2)
                C.append(c)

            def front2(c):
                s1, s2 = c["ins"]
                w = c["w"]
                lastrd.append(
                    nc.vector.tensor_tensor(c["S"][:, :w], s1, s2, Op.add))

            def absd(c):  # ACT: |dx|, |dy|, |D| and the scaled squares,
                # reading the interleaved m3 columns (stride-blind engine)
                w = c["w"]
                d3 = c["m3"][:, : 3 * w].rearrange("p (w c) -> p w c", c=3)
                dx, dy, D = d3[:, :, 0], d3[:, :, 1], d3[:, :, 2]
                nc.scalar.activation(c["adx"][:, :w], dx, Act.Abs)
                nc.scalar.activation(c["ady"][:, :w], dy, Act.Abs)
                nc.scalar.activation(c["aD"][:, :w], D, Act.Abs)
                nc.scalar.activation(c["qS"][:, :w], c["S"][:, :w], Act.Square,
                                     scale=RT2)
                nc.scalar.activation(c["qD"][:, :w], D, Act.Square,
                                     scale=RT2)

            def mids(c):  # DVE: thresholds (fp16 2x), w-extent + its relu
                w = c["w"]
                nc.vector.tensor_tensor(c["mw"][:, :w], c["adx"][:, :w],
                                        c["aD"][:, :w], Op.max)
                nc.vector.tensor_tensor(c["mh"][:, :w], c["ady"][:, :w],
                                        c["aD"][:, :w], Op.max)
                nc.vector.tensor_sub(c["wr"][:, :w], c["S"][:, :w], c["mw"][:, :w])
                nc.vector.tensor_scalar_max(c["wr"][:, :w], c["wr"][:, :w], 0.0)

            def hrst(c):  # Pool: hr = S - mh
                w = c["w"]
                nc.gpsimd.tensor_tensor(c["hr"][:, :w], c["S"][:, :w],
                                        c["mh"][:, :w], Op.subtract)

            def relus(c):  # ACT: clamp the h-extent
                w = c["w"]
                nc.scalar.activation(c["rh"][:, :w], c["hr"][:, :w], Act.Relu)

            def ovst(c):  # Pool: ov = relu(wr)*rh
                w = c["w"]
                nc.gpsimd.tensor_tensor(c["ov"][:, :w], c["wr"][:, :w],
                                        c["rh"][:, :w], Op.mult)

            def uemm(c):  # PE: ue = eps + 2S^2 + 2D^2 - ov in PSUM
                for bank, o in c["banks"]:
                    nc.tensor.matmul(bank[:, :], epsw[:, :], ones2[:, :],
                                     start=True, stop=False)
                    nc.tensor.matmul(bank[:, :], ident[:, :],
                                     c["qS"][:, o : o + H], start=False, stop=False)
                    nc.tensor.matmul(bank[:, :], ident[:, :],
                                     c["qD"][:, o : o + H], start=False, stop=False)
                    nc.tensor.matmul(bank[:, :], nident[:, :],
                                     c["ov"][:, o : o + H], start=False, stop=True)

            def recip(c):  # DVE: r = 1/(u+eps), ~18 bits, straight off PSUM
                for bank, o in c["banks"]:
                    nc.vector.reciprocal_approx_fast(c["r"][:, o : o + H],
                                                     bank[:, :])

            def ioust(c):  # Pool: iou = ov * r (fp16 * fp32 -> fp16)
                w = c["w"]
                nc.gpsimd.tensor_tensor(c["iou"][:, :w], c["ov"][:, :w],
                                        c["r"][:, :w], Op.mult)

            def iou_psum(c):  # PE: per-seg partition-sums into PSUM banks
                t, w = c["t"], c["w"]
                nc.tensor.matmul(psA1[:, :], ones[:, :], c["iou"][:, :H],
                                 start=(t == 0), stop=(t == NSEG - 1))
                if w > H:
                    nc.tensor.matmul(psA2[:, :], ones[:, :], c["iou"][:, H : 2 * H],
                                     start=(t == BIG_LO), stop=(t == BIG_HI))

            def accum(c):  # ACT: loss partial rides the Ln accumulator
                t, w = c["t"], c["w"]
                nc.scalar.activation(
                    c["sc"][:, :w], c["iou"][:, :w], Act.Ln,
                    bias=eps_t[:, 0:1],
                    accum_out=accs[:, t : t + 1],
                )

            def ps_extract(bank, col, n):
                pscp = pool.tile([1, H], F32, tag="pscp", name=f"pscp_{col}")
                nc.scalar.copy(pscp[:, :n], bank[:, :n])
                nc.sync.dma_start(out=iouv_d[:, col : col + n],
                                  in_=pscp[:, :n])

            # Six-stage pipeline; per-iteration engine queue orders:
            #   ACT : ln(k-6) | rh(k-3) | abs,squares(k-1)
            #   Pool: iou(k-5) | ov(k-3) | hr(k-2)
            #   PE  : iou_psum(k-6) | uemm(k-3) | recycle drain(k)
            #   DVE : mw,mh,wr,relu(k-2) | recip(k-4) | dummy,m3(k) | S(k)
            for k in range(NSEG + 7):
                if 6 <= k <= NSEG + 5:
                    accum(C[k - 6])
                    iou_psum(C[k - 6])
                if 5 <= k <= NSEG + 4:
                    ioust(C[k - 5])
                if 4 <= k <= NSEG + 3:
                    # recip(s) must be ISSUED before uemm(s+1): both touch
                    # the shared ueP banks, and program order decides
                    # whether the framework sees a (correct) WAR or a
                    # (wrong-generation) RAW dependency.
                    recip(C[k - 4])
                if 3 <= k <= NSEG + 2:
                    relus(C[k - 3])
                    ovst(C[k - 3])
                    uemm(C[k - 3])
                if 2 <= k <= NSEG + 1:
                    mids(C[k - 2])
                if k < NSEG:
                    front(k)
                if 2 <= k <= NSEG + 1:
                    hrst(C[k - 2])
                if 1 <= k <= NSEG:
                    absd(C[k - 1])
                if k < NSEG:
                    front2(C[k])
                if k == NSEG + 5:  # psA2 closed at iter NSEG+4 (seg BIG_HI)
                    ps_extract(psA2, H, H)

            ps_extract(psA1, 0, H)
            nc.sync.dma_start(out=acc_d[:, :], in_=accs[:, :])

    if compile_passes:
        nc.compile()
    return nc


_NC_CACHE: list[bass.Bass] = []


def _get_nc() -> bass.Bass:
    if not _NC_CACHE:
        _NC_CACHE.append(_build())
    return _NC_CACHE[0]


def _run(inputs: dict, trace: bool = False, trace_kwargs: dict | None = None):
    outputs = np.ascontiguousarray(np.asarray(inputs["outputs"], dtype=np.float32))
    targets = np.ascontiguousarray(np.asarray(inputs["targets"], dtype=np.float32))
    assert outputs.shape == (N, 3) and targets.shape == (N, 3)

    nc = _get_nc()
    in_maps = [
        {
            "outputs": outputs[c * NS : (c + 1) * NS],
            "targets": targets[c * NS : (c + 1) * NS],
        }
        for c in range(NCORES)
    ]
    kw = {}
    if trace:
        kw["trace"] = True
        if trace_kwargs:
            kw["trace_kwargs"] = trace_kwargs
    res = run_bass_kernel_spmd(nc, in_maps, list(range(NCORES)), **kw)

    iou_sum = 0.0
    loss = 0.0
    for c in range(NCORES):
        acc = np.asarray(res.results[c]["acc"], dtype=np.float64)
        loss += acc.sum()
        iou_sum += np.asarray(res.results[c]["iouv"], dtype=np.float64).sum()
    loss = -loss
    return (np.float32(loss), np.float32(iou_sum)), res


def kernel(**inputs) -> tuple:
    (loss, iou_sum), _ = _run(inputs)
    return (loss, iou_sum)


# revision 22
# speedup vs baseline: 1.2430x; 1.0281x over previous
"""IoU loss kernel for Trainium2, data-parallel over 8 NeuronCores.

Math (per box, columns = x-center, y-center, half-size s):
    w = relu(min(x+s, x'+s') - max(x-s, x'-s'))
      = relu((s+s') - max(|x-x'|, |s-s'|))          # S - max identity
    h likewise with y.
    overlap = w*h
    union   = 4s^2 + 4s'^2 - overlap = 2(S^2 + D^2) - overlap,
              S = s+s', D = s-s'
    iou     = overlap / (union + 1e-7)
    loss    = -sum(log(iou + 1e-7));  iou_sum = sum(iou)

The DMA stream (two fp32 loads per tile, 24 MiB/core total, ~70us at
the 358 GB/s per-core HBM rate) and DVE/ACT are co-bottlenecks; the
work is spread so no engine exceeds ~8.5us per KTile segment. The key
SBUF-bandwidth trick: four strided fp32 reads of the AoS raw data
would fetch the same 32B beats four times (~98KB/partition/seg);
instead ONE contiguous fp32 subtract m3 = rawO - rawT covers dx, dy
and D at full beat utilization, and the column extraction rides the
stride-blind ACT ops. Only S = s1+s2 stays strided.

  DVE  (~8.4us/KTile): m3 (contiguous fp32 -> fp16, 3W cols), S
         (strided), mw/mh = max(a.,aD) (fp16 2x), wr = S - mw,
         relu(wr) (tensor_scalar 4x), r = 1/(u+eps) via
         reciprocal_approx_fast (~18 bits).
  ACT  (~8.2us/KTile): |dx|, |dy|, |D| (Abs on m3's strided columns),
         relu(hr), 2S^2/2D^2 (Square with scale=sqrt2), ue32 =
         Relu(ue_psum)+eps upcast (2 half-tiles from PSUM),
         Ln(iou+eps) accum -> loss partial. All funcs live in the
         `natural_log` table set -> ONE table load total.
  Pool (~7.5us/KTile): hr = S - mh, ov = rw*rh, iou = ov*r (the three
         Q7-ucode tensor_tensor slots; mixed fp16*fp32 iou mult).
  PE   (~4.5us/KTile): ue = I*qS + I*qD + (-I)*ov accumulated in PSUM
         (identity-weight matmuls; removes the q12/ue16 elementwise
         chain from DVE/Pool), plus ones-weight matmuls accumulating
         the per-partition iou sums across all segments into two PSUM
         banks. PE drains also absorb the raw-slot recycle waits so
         the DMA stream stays wait-free.

Six-iteration software pipeline; the per-iteration engine queue orders
put ready work first so cross-engine deps are either >=1 iteration old
or produced earlier in the same iteration by an engine ahead in its
own queue. Trailing segments shrink (512/512) to cut the drain-out
tail. Host: final [128, NSEG] x 8 cores loss-partial reduction +
[1,1024] iou partials, summed in float64.
"""

import numpy as np

import concourse.bass as bass
import concourse.mybir as mybir
from concourse import tile
from concourse.bass_utils import run_bass_kernel_spmd

N = 8388608
NCORES = 8
NS = N // NCORES  # 1048576 boxes per core
P = 128
W = 1024          # boxes per partition per full tile
T = NS // (P * W)  # 8 full-tile units per core
EPS = 1e-7
RT2 = 1.4142135623730951

F32 = mybir.dt.float32
F16 = mybir.dt.float16
Op = mybir.AluOpType
Act = mybir.ActivationFunctionType


def _build(T_: int = T, W_: int = W, compile_passes: bool = True) -> bass.Bass:
    from concourse import bacc
    from concourse.tile_rust import add_dep_helper

    # small leading segment cuts the pipeline-fill stall (DVE waits on
    # the first DMA); small trailing segment cuts the drain-out tail.
    segs = [W_ // 2] + [W_] * (T_ - 1) + [W_ // 2]
    assert sum(segs) == T_ * W_
    NSEG = len(segs)
    SMALLW = W_ // 2
    H = W_ // 2  # psum bank width (512 fp32 cols)
    BIG_LO, BIG_HI = 1, T_ - 1  # segs with width W_ (inclusive range)

    ns = P * W_ * T_
    nc = bacc.Bacc()
    outs_d = nc.dram_tensor("outputs", [ns, 3], F32, kind="ExternalInput")
    tars_d = nc.dram_tensor("targets", [ns, 3], F32, kind="ExternalInput")
    acc_d = nc.dram_tensor("acc", [P, NSEG], F32, kind="ExternalOutput")
    iouv_d = nc.dram_tensor("iouv", [1, 2 * H], F32, kind="ExternalOutput")

    offs = [0]
    for w in segs:
        offs.append(offs[-1] + w)

    def seg_view(dram, s):
        b0 = P * offs[s]
        return dram[b0 : b0 + P * segs[s], :].rearrange(
            "(p w) c -> p (w c)", p=P, w=segs[s]
        )

    RAWBUFS = 3

    with tile.TileContext(nc) as tc:
        with (tc.tile_pool(name="main", bufs=2) as pool,
              tc.tile_pool(name="psum", bufs=1,
                           space=bass.MemorySpace.PSUM) as psum):
            accs = pool.tile([P, NSEG], F32, tag="accs", bufs=1)
            nc.vector.memset(accs[:, :], 0.0)
            eps_t = pool.tile([P, 1], F32, tag="eps", bufs=1)
            nc.vector.memset(eps_t[:, :], EPS)
            ones = pool.tile([P, 1], F16, tag="ones", bufs=1)
            nc.vector.memset(ones[:, :], 1.0)

            # identity / -identity fp16 weights for the PSUM union
            # accumulation: iota(col - partition) is exact in fp16
            # (range +-127), is_equal against 0 marks the diagonal.
            colmp = pool.tile([P, P], F16, tag="colmp", bufs=1)
            nc.gpsimd.iota(colmp[:, :], [[1, P]], channel_multiplier=-1,
                           allow_small_or_imprecise_dtypes=True)
            ident = pool.tile([P, P], F16, tag="ident", bufs=1)
            nc.vector.tensor_scalar(ident[:, :], colmp[:, :], 0.0, None,
                                    Op.is_equal)
            nident = pool.tile([P, P], F16, tag="nident", bufs=1)
            nc.vector.tensor_scalar(nident[:, :], colmp[:, :], 0.0, -1.0,
                                    Op.is_equal, Op.mult)

            # eps enters the union inside PSUM: a bf16 rank-full matmul
            # epsw.T @ ones2 adds 128 * (eps/128) = eps to every element,
            # so the reciprocal can read PSUM directly (no ACT upcast op).
            BF16 = mybir.dt.bfloat16
            epsw = pool.tile([P, P], BF16, tag="epsw", bufs=1)
            nc.vector.memset(epsw[:, :], EPS / P)
            ones2 = pool.tile([P, H], BF16, tag="ones2", bufs=1)
            nc.vector.memset(ones2[:, :], 1.0)

            # PSUM: iou partition-sum accumulators (whole kernel) plus the
            # union banks. The union banks are single-buffered and shared
            # across segments: issuing recip(s) BEFORE uemm(s+1) makes the
            # framework serialize the next segment's matmuls behind the
            # previous reciprocal read (WAR) — PE has slack to absorb it.
            psA1 = psum.tile([1, H], F32, tag="psA1", bufs=1)
            psA2 = psum.tile([1, H], F32, tag="psA2", bufs=1)
            ueP1 = psum.tile([P, H], F32, tag="ueP1", bufs=1)
            ueP2 = psum.tile([P, H], F32, tag="ueP2", bufs=1)

            lastrd: list = []
            dmaO_h: list = []
            dmaT_h: list = []
            big_idx: list = []
            C: list = []

            def front(t):
                w = segs[t]
                if w <= SMALLW:
                    rawO = pool.tile([P, 3 * SMALLW], F32, tag="rawOs", bufs=3)
                    rawT = pool.tile([P, 3 * SMALLW], F32, tag="rawTs", bufs=3)
                    recycle = None
                else:
                    rawO = pool.tile([P, 3 * W_], F32, tag="rawO", bufs=RAWBUFS)
                    rawT = pool.tile([P, 3 * W_], F32, tag="rawT", bufs=RAWBUFS)
                    nbig = len(big_idx)
                    recycle = big_idx[nbig - RAWBUFS] if nbig >= RAWBUFS else None
                    big_idx.append(t)
                deps = []
                if recycle is not None:
                    dr = nc.tensor.drain(fusable=False)
                    add_dep_helper(dr.ins, lastrd[recycle].ins, sync=True,
                                   reason="absorb DVE WAR tick")
                    add_dep_helper(dr.ins, dmaO_h[recycle].ins, sync=True,
                                   reason="absorb old rawO DMA lane")
                    add_dep_helper(dr.ins, dmaT_h[recycle].ins, sync=True,
                                   reason="absorb old rawT DMA lane")
                    deps = [dr]
                dmaO = nc.sync.dma_start(out=rawO[:, : 3 * w], in_=seg_view(outs_d, t))
                dmaT = nc.sync.dma_start(out=rawT[:, : 3 * w], in_=seg_view(tars_d, t))
                for d in deps:
                    add_dep_helper(dmaO.ins, d.ins, sync=True,
                                   reason="slot guarded by PE drain")
                    add_dep_helper(dmaT.ins, d.ins, sync=True,
                                   reason="slot guarded by PE drain")
                dmaO_h.append(dmaO)
                dmaT_h.append(dmaT)

                # m3 depends on BOTH input DMAs but has one sync-wait
                # slot; absorb rawT's semaphore with a tiny same-queue copy.
                dummy = pool.tile([P, 1], F32, tag="dummy")
                nc.vector.tensor_copy(dummy[:, :], rawT[:, 0:1])
                o3 = rawO[:, : 3 * w].rearrange("p (w c) -> p w c", c=3)
                t3 = rawT[:, : 3 * w].rearrange("p (w c) -> p w c", c=3)
                s1, s2 = o3[:, :, 2], t3[:, :, 2]

                c = {"t": t, "w": w}
                c["m3"] = pool.tile([P, 3 * W_], F16, tag="m3", bufs=2,
                                    name=f"m3_{t}")
                for nm, nb in (("S", 3),
                               ("adx", 2), ("ady", 2), ("aD", 2),
                               ("mw", 2), ("mh", 2), ("wr", 2), ("hr", 2),
                               ("rw", 2), ("rh", 2), ("qS", 3), ("qD", 3),
                               ("ov", 2), ("iou", 2), ("sc", 1)):
                    c[nm] = pool.tile([P, W_], F16, tag=nm, bufs=nb,
                                      name=f"{nm}_{t}")
                c["r"] = pool.tile([P, W_], F32, tag="r", name=f"r_{t}")
                c["banks"] = ([(ueP1, 0)] if w <= H
                              else [(ueP1, 0), (ueP2, H)])

                # the whole AoS de-interleave collapses into ONE contiguous
                # fp32 subtract (full 32B-beat utilization); the strided
                # column extraction happens on ACT, which is stride-blind.
                nc.vector.tensor_tensor(c["m3"][:, : 3 * w],
                                        rawO[:, : 3 * w], rawT[:, : 3 * w],
                                        Op.subtract)
                c["ins"] = (s1, s2)
                C.append(c)

            def front2(c):
                s1, s2 = c["ins"]
                w = c["w"]
                lastrd.append(
                    nc.vector.tensor_tensor(c["S"][:, :w], s1, s2, Op.add))

            def absd(c):  # ACT: |dx|, |dy|, |D| and the scaled squares,
                # reading the interleaved m3 columns (stride-blind engine)
                w = c["w"]
                d3 = c["m3"][:, : 3 * w].rearrange("p (w c) -> p w c", c=3)
                dx, dy, D = d3[:, :, 0], d3[:, :, 1], d3[:, :, 2]
                nc.scalar.activation(c["adx"][:, :w], dx, Act.Abs)
                nc.scalar.activation(c["ady"][:, :w], dy, Act.Abs)
                nc.scalar.activation(c["aD"][:, :w], D, Act.Abs)
                nc.scalar.activation(c["qS"][:, :w], c["S"][:, :w], Act.Square,
                                     scale=RT2)
                nc.scalar.activation(c["qD"][:, :w], D, Act.Square,
                                     scale=RT2)

            def mids(c):  # DVE: thresholds (fp16 2x), w-extent + its relu
                w = c["w"]
                nc.vector.tensor_tensor(c["mw"][:, :w], c["adx"][:, :w],
                                        c["aD"][:, :w], Op.max)
                nc.vector.tensor_tensor(c["mh"][:, :w], c["ady"][:, :w],
                                        c["aD"][:, :w], Op.max)
                nc.vector.tensor_sub(c["wr"][:, :w], c["S"][:, :w], c["mw"][:, :w])

            def hrst(c):  # Pool: hr = S - mh
                w = c["w"]
                nc.gpsimd.tensor_tensor(c["hr"][:, :w], c["S"][:, :w],
                                        c["mh"][:, :w], Op.subtract)

            def relus(c):  # ACT: clamp both extents
                w = c["w"]
                nc.scalar.activation(c["rw"][:, :w], c["wr"][:, :w], Act.Relu)
                nc.scalar.activation(c["rh"][:, :w], c["hr"][:, :w], Act.Relu)

            def ovst(c):  # Pool: ov = rw*rh
                w = c["w"]
                nc.gpsimd.tensor_tensor(c["ov"][:, :w], c["rw"][:, :w],
                                        c["rh"][:, :w], Op.mult)

            def uemm(c):  # PE: ue = eps + 2S^2 + 2D^2 - ov in PSUM
                for bank, o in c["banks"]:
                    nc.tensor.matmul(bank[:, :], epsw[:, :], ones2[:, :],
                                     start=True, stop=False)
                    nc.tensor.matmul(bank[:, :], ident[:, :],
                                     c["qS"][:, o : o + H], start=False, stop=False)
                    nc.tensor.matmul(bank[:, :], ident[:, :],
                                     c["qD"][:, o : o + H], start=False, stop=False)
                    nc.tensor.matmul(bank[:, :], nident[:, :],
                                     c["ov"][:, o : o + H], start=False, stop=True)

            def recip(c):  # DVE: r = 1/(u+eps), ~18 bits, straight off PSUM
                for bank, o in c["banks"]:
                    nc.vector.reciprocal_approx_fast(c["r"][:, o : o + H],
                                                     bank[:, :])

            def ioust(c):  # Pool: iou = ov * r (fp16 * fp32 -> fp16)
                w = c["w"]
                nc.gpsimd.tensor_tensor(c["iou"][:, :w], c["ov"][:, :w],
                                        c["r"][:, :w], Op.mult)

            def iou_psum(c):  # PE: per-seg partition-sums into PSUM banks
                t, w = c["t"], c["w"]
                nc.tensor.matmul(psA1[:, :], ones[:, :], c["iou"][:, :H],
                                 start=(t == 0), stop=(t == NSEG - 1))
                if w > H:
                    nc.tensor.matmul(psA2[:, :], ones[:, :], c["iou"][:, H : 2 * H],
                                     start=(t == BIG_LO), stop=(t == BIG_HI))

            def accum(c):  # ACT: loss partial rides the Ln accumulator
                t, w = c["t"], c["w"]
                nc.scalar.activation(
                    c["sc"][:, :w], c["iou"][:, :w], Act.Ln,
                    bias=eps_t[:, 0:1],
                    accum_out=accs[:, t : t + 1],
                )

            def ps_extract(bank, col, n):
                pscp = pool.tile([1, H], F32, tag="pscp", name=f"pscp_{col}")
                nc.scalar.copy(pscp[:, :n], bank[:, :n])
                nc.sync.dma_start(out=iouv_d[:, col : col + n],
                                  in_=pscp[:, :n])

            # Six-stage pipeline; per-iteration engine queue orders:
            #   ACT : ln(k-6) | rh(k-3) | abs,squares(k-1)
            #   Pool: iou(k-5) | ov(k-3) | hr(k-2)
            #   PE  : iou_psum(k-6) | uemm(k-3) | recycle drain(k)
            #   DVE : mw,mh,wr,relu(k-2) | recip(k-4) | dummy,m3(k) | S(k)
            for k in range(NSEG + 7):
                if 6 <= k <= NSEG + 5:
                    accum(C[k - 6])
                    iou_psum(C[k - 6])
                if 5 <= k <= NSEG + 4:
                    ioust(C[k - 5])
                if 4 <= k <= NSEG + 3:
                    # recip(s) must be ISSUED before uemm(s+1): both touch
                    # the shared ueP banks, and program order decides
                    # whether the framework sees a (correct) WAR or a
                    # (wrong-generation) RAW dependency.
                    recip(C[k - 4])
                if 3 <= k <= NSEG + 2:
                    relus(C[k - 3])
                    ovst(C[k - 3])
                    uemm(C[k - 3])
                if 2 <= k <= NSEG + 1:
                    mids(C[k - 2])
                if k < NSEG:
                    front(k)
                if 2 <= k <= NSEG + 1:
                    hrst(C[k - 2])
                if 1 <= k <= NSEG:
                    absd(C[k - 1])
                if k < NSEG:
                    front2(C[k])
                if k == NSEG + 5:  # psA2 closed at iter NSEG+4 (seg BIG_HI)
                    ps_extract(psA2, H, H)

            ps_extract(psA1, 0, H)
            nc.sync.dma_start(out=acc_d[:, :], in_=accs[:, :])

    if compile_passes:
        nc.compile()
    return nc


_NC_CACHE: list[bass.Bass] = []


def _get_nc() -> bass.Bass:
    if not _NC_CACHE:
        _NC_CACHE.append(_build())
    return _NC_CACHE[0]


def _run(inputs: dict, trace: bool = False, trace_kwargs: dict | None = None):
    outputs = np.ascontiguousarray(np.asarray(inputs["outputs"], dtype=np.float32))
    targets = np.ascontiguousarray(np.asarray(inputs["targets"], dtype=np.float32))
    assert outputs.shape == (N, 3) and targets.shape == (N, 3)

    nc = _get_nc()
    in_maps = [
        {
            "outputs": outputs[c * NS : (c + 1) * NS],
            "targets": targets[c * NS : (c + 1) * NS],
        }
        for c in range(NCORES)
    ]
    kw = {}
    if trace:
        kw["trace"] = True
        if trace_kwargs:
            kw["trace_kwargs"] = trace_kwargs
    res = run_bass_kernel_spmd(nc, in_maps, list(range(NCORES)), **kw)

    iou_sum = 0.0
    loss = 0.0
    for c in range(NCORES):
        acc = np.asarray(res.results[c]["acc"], dtype=np.float64)
        loss += acc.sum()
        iou_sum += np.asarray(res.results[c]["iouv"], dtype=np.float64).sum()
    loss = -loss
    return (np.float32(loss), np.float32(iou_sum)), res


def kernel(**inputs) -> tuple:
    (loss, iou_sum), _ = _run(inputs)
    return (loss, iou_sum)


# revision 25
# speedup vs baseline: 1.3695x; 1.1018x over previous
"""IoU loss kernel for Trainium2, data-parallel over 8 NeuronCores.

Math (per box, columns = x-center, y-center, half-size s):
    w = relu(min(x+s, x'+s') - max(x-s, x'-s'))
      = relu((s+s') - max(|x-x'|, |s-s'|))          # S - max identity
    h likewise with y.
    overlap = w*h
    union   = 4s^2 + 4s'^2 - overlap = 2(S^2 + D^2) - overlap,
              S = s+s', D = s-s'
    iou     = overlap / (union + 1e-7)
    loss    = -sum(log(iou + 1e-7));  iou_sum = sum(iou)

Engine-assignment rationale. The SBUF fabric saturates when several
engines stream wide operands concurrently, and DVE instructions (the
in-order bottleneck queue) inflate 1.5-2.3x under that load; PE
matmuls never inflate (private weight/moving-data path + PSUM
output). So everything that CAN be a matmul IS one:

  PE   : SP = I*s1 + I*s2 (fp32 identity matmuls straight off the
         interleaved raw fp32 columns -> PSUM, replacing a DVE strided
         add that measured 3.5us/seg), ue = eps + 2S^2 + 2D^2 - ov
         accumulated in PSUM (eps enters via a bf16 rank-full matmul
         epsw.T @ ones2 = 128 * eps/128), and the ones-weight matmuls
         accumulating per-partition iou sums across all segments. PE
         drains also absorb the raw-slot recycle waits.
  DVE  (~9.8us/KTile): m3 = rawO - rawT, ONE contiguous fp32 subtract
         covering dx, dy, D (four strided reads would re-fetch the
         same 32B beats four times), mw/mh = max(a., aD) (fp16 2x),
         wr/hr = SP - m. (PSUM-operand tensor_tensor), and
         r = 1/(u+eps) via reciprocal_approx_fast off PSUM.
  ACT  (~8.3us/KTile): |dx|, |dy|, |D| (Abs on m3's strided columns -
         ACT is stride-blind), 2D^2/2S^2 (Square with scale=sqrt2; qS
         reads the SP PSUM banks), relu(wr)/relu(hr), Ln(iou+eps)
         accum -> loss partial. All funcs live in the `natural_log`
         table set -> ONE table load total.
  Pool (~5.2us/KTile): ov = rw*rh, iou = ov*r (fp16*fp32), the two
         Q7-ucode tensor_tensor slots.

PSUM discipline: the SP and ue banks are single-buffered and shared
across segments; every reader of generation s is ISSUED before the
generation s+1 matmuls so the tile framework sees a WAR (serialize
correctly) instead of a wrong-generation RAW. PE has slack to absorb
the resulting waits.

Six-iteration software pipeline; trailing/leading segments shrink
(512) to cut fill and drain. Host: final [128, NSEG] x 8 cores loss
partials + [1,1024] iou partials, summed in float64.
"""

import numpy as np

import concourse.bass as bass
import concourse.mybir as mybir
from concourse import tile
from concourse.bass_utils import run_bass_kernel_spmd

N = 8388608
NCORES = 8
NS = N // NCORES  # 1048576 boxes per core
P = 128
W = 1024          # boxes per partition per full tile
T = NS // (P * W)  # 8 full-tile units per core
EPS = 1e-7
RT2 = 1.4142135623730951

F32 = mybir.dt.float32
F16 = mybir.dt.float16
Op = mybir.AluOpType
Act = mybir.ActivationFunctionType


def _build(T_: int = T, W_: int = W, compile_passes: bool = True) -> bass.Bass:
    from concourse import bacc
    from concourse.tile_rust import add_dep_helper

    # small leading segment cuts the pipeline-fill stall (DVE waits on
    # the first DMA); small trailing segment cuts the drain-out tail.
    segs = [W_ // 2] + [W_] * (T_ - 1) + [W_ // 2]
    assert sum(segs) == T_ * W_
    NSEG = len(segs)
    SMALLW = W_ // 2
    H = W_ // 2  # psum bank width (512 fp32 cols)
    BIG_LO, BIG_HI = 1, T_ - 1  # segs with width W_ (inclusive range)

    ns = P * W_ * T_
    nc = bacc.Bacc()
    outs_d = nc.dram_tensor("outputs", [ns, 3], F32, kind="ExternalInput")
    tars_d = nc.dram_tensor("targets", [ns, 3], F32, kind="ExternalInput")
    acc_d = nc.dram_tensor("acc", [P, NSEG], F32, kind="ExternalOutput")
    iouv_d = nc.dram_tensor("iouv", [1, 2 * H], F32, kind="ExternalOutput")

    offs = [0]
    for w in segs:
        offs.append(offs[-1] + w)

    def seg_view(dram, s):
        b0 = P * offs[s]
        return dram[b0 : b0 + P * segs[s], :].rearrange(
            "(p w) c -> p (w c)", p=P, w=segs[s]
        )

    RAWBUFS = 3

    with tile.TileContext(nc) as tc:
        with (tc.tile_pool(name="main", bufs=2) as pool,
              tc.tile_pool(name="psum", bufs=1,
                           space=bass.MemorySpace.PSUM) as psum):
            accs = pool.tile([P, NSEG], F32, tag="accs", bufs=1)
            nc.vector.memset(accs[:, :], 0.0)
            eps_t = pool.tile([P, 1], F32, tag="eps", bufs=1)
            nc.vector.memset(eps_t[:, :], EPS)
            ones = pool.tile([P, 1], F16, tag="ones", bufs=1)
            nc.vector.memset(ones[:, :], 1.0)

            # identity / -identity weights: iota(col - partition) is exact
            # in fp16 (range +-127), is_equal against 0 marks the diagonal.
            colmp = pool.tile([P, P], F16, tag="colmp", bufs=1)
            nc.gpsimd.iota(colmp[:, :], [[1, P]], channel_multiplier=-1,
                           allow_small_or_imprecise_dtypes=True)
            ident = pool.tile([P, P], F16, tag="ident", bufs=1)
            nc.vector.tensor_scalar(ident[:, :], colmp[:, :], 0.0, None,
                                    Op.is_equal)
            nident = pool.tile([P, P], F16, tag="nident", bufs=1)
            nc.vector.tensor_scalar(nident[:, :], colmp[:, :], 0.0, -1.0,
                                    Op.is_equal, Op.mult)
            identF = pool.tile([P, P], F32, tag="identF", bufs=1)
            nc.vector.tensor_copy(identF[:, :], ident[:, :])

            # eps enters the union inside PSUM: a bf16 rank-full matmul
            # epsw.T @ ones2 adds 128 * (eps/128) = eps to every element.
            BF16 = mybir.dt.bfloat16
            epsw = pool.tile([P, P], BF16, tag="epsw", bufs=1)
            nc.vector.memset(epsw[:, :], EPS / P)
            ones2 = pool.tile([P, H], BF16, tag="ones2", bufs=1)
            nc.vector.memset(ones2[:, :], 1.0)

            # PSUM: iou partition-sum accumulators (whole kernel) plus the
            # shared single-buffered SP (s1+s2) and ue banks; see module
            # docstring for the program-order WAR discipline.
            psA1 = psum.tile([1, H], F32, tag="psA1", bufs=1)
            psA2 = psum.tile([1, H], F32, tag="psA2", bufs=1)
            ueP1 = psum.tile([P, H], F32, tag="ueP1", bufs=1)
            ueP2 = psum.tile([P, H], F32, tag="ueP2", bufs=1)
            # SP banks alternate by segment parity: spmm(s+1) is issued
            # (iteration s+1) before the segment-s extents are read
            # (iteration s+2), so a single shared pair would hand mids the
            # wrong generation. Two pairs give 2-segment WAR spacing.
            SPa1 = psum.tile([P, H], F32, tag="SPa1", bufs=1)
            SPa2 = psum.tile([P, H], F32, tag="SPa2", bufs=1)
            SPb1 = psum.tile([P, H], F32, tag="SPb1", bufs=1)
            SPb2 = psum.tile([P, H], F32, tag="SPb2", bufs=1)

            lastrd: list = []
            spmm_h: list = []
            dmaO_h: list = []
            dmaT_h: list = []
            big_idx: list = []
            C: list = []

            def front(t):
                w = segs[t]
                if w <= SMALLW:
                    rawO = pool.tile([P, 3 * SMALLW], F32, tag="rawOs", bufs=3)
                    rawT = pool.tile([P, 3 * SMALLW], F32, tag="rawTs", bufs=3)
                    recycle = None
                else:
                    rawO = pool.tile([P, 3 * W_], F32, tag="rawO", bufs=RAWBUFS)
                    rawT = pool.tile([P, 3 * W_], F32, tag="rawT", bufs=RAWBUFS)
                    nbig = len(big_idx)
                    recycle = big_idx[nbig - RAWBUFS] if nbig >= RAWBUFS else None
                    big_idx.append(t)
                deps = []
                if recycle is not None:
                    dr = nc.tensor.drain(fusable=False)
                    add_dep_helper(dr.ins, lastrd[recycle].ins, sync=True,
                                   reason="absorb DVE WAR tick")
                    for mm in spmm_h[recycle]:
                        add_dep_helper(dr.ins, mm.ins, sync=True,
                                       reason="absorb PE raw-read WAR")
                    add_dep_helper(dr.ins, dmaO_h[recycle].ins, sync=True,
                                   reason="absorb old rawO DMA lane")
                    add_dep_helper(dr.ins, dmaT_h[recycle].ins, sync=True,
                                   reason="absorb old rawT DMA lane")
                    deps = [dr]
                dmaO = nc.sync.dma_start(out=rawO[:, : 3 * w], in_=seg_view(outs_d, t))
                dmaT = nc.sync.dma_start(out=rawT[:, : 3 * w], in_=seg_view(tars_d, t))
                for d in deps:
                    add_dep_helper(dmaO.ins, d.ins, sync=True,
                                   reason="slot guarded by PE drain")
                    add_dep_helper(dmaT.ins, d.ins, sync=True,
                                   reason="slot guarded by PE drain")
                dmaO_h.append(dmaO)
                dmaT_h.append(dmaT)

                # m3 depends on BOTH input DMAs but has one sync-wait
                # slot; absorb rawT's semaphore with a tiny same-queue copy.
                dummy = pool.tile([P, 1], F32, tag="dummy")
                nc.vector.tensor_copy(dummy[:, :], rawT[:, 0:1])

                c = {"t": t, "w": w, "rawO": rawO, "rawT": rawT}
                c["m3"] = pool.tile([P, 3 * W_], F16, tag="m3", bufs=2,
                                    name=f"m3_{t}")
                for nm, nb in (("adx", 2), ("ady", 2), ("aD", 2),
                               ("mw", 2), ("mh", 2), ("wr", 2), ("hr", 2),
                               ("rw", 2), ("rh", 2), ("qS", 3), ("qD", 3),
                               ("ov", 2), ("iou", 2), ("sc", 1)):
                    c[nm] = pool.tile([P, W_], F16, tag=nm, bufs=nb,
                                      name=f"{nm}_{t}")
                c["r"] = pool.tile([P, W_], F32, tag="r", name=f"r_{t}")
                c["ue_banks"] = ([(ueP1, 0)] if w <= H
                                 else [(ueP1, 0), (ueP2, H)])
                b1, b2 = (SPa1, SPa2) if t % 2 == 0 else (SPb1, SPb2)
                c["sp_banks"] = [(b1, 0)] if w <= H else [(b1, 0), (b2, H)]

                # the whole AoS de-interleave collapses into ONE contiguous
                # fp32 subtract (full 32B-beat utilization); the strided
                # column extraction happens on ACT, which is stride-blind.
                lastrd.append(
                    nc.vector.tensor_tensor(c["m3"][:, : 3 * w],
                                            rawO[:, : 3 * w], rawT[:, : 3 * w],
                                            Op.subtract))
                C.append(c)

            def spmm(c):  # PE: SP = s1 + s2 off the raw strided fp32 cols
                w = c["w"]
                o1 = c["rawO"][:, : 3 * w].rearrange("p (w c) -> p w c", c=3)
                t1 = c["rawT"][:, : 3 * w].rearrange("p (w c) -> p w c", c=3)
                s1, s2 = o1[:, :, 2], t1[:, :, 2]
                hs = []
                for bank, o in c["sp_banks"]:
                    hs.append(nc.tensor.matmul(bank[:, :], identF[:, :],
                                               s1[:, o : o + H],
                                               start=True, stop=False))
                    hs.append(nc.tensor.matmul(bank[:, :], identF[:, :],
                                               s2[:, o : o + H],
                                               start=False, stop=True))
                spmm_h.append(hs)

            def absd(c):  # ACT: |dx|, |dy|, |D| off m3's interleaved
                # columns, 2D^2 likewise, 2S^2 off the SP PSUM banks
                w = c["w"]
                d3 = c["m3"][:, : 3 * w].rearrange("p (w c) -> p w c", c=3)
                dx, dy, D = d3[:, :, 0], d3[:, :, 1], d3[:, :, 2]
                nc.scalar.activation(c["adx"][:, :w], dx, Act.Abs)
                nc.scalar.activation(c["ady"][:, :w], dy, Act.Abs)
                nc.scalar.activation(c["aD"][:, :w], D, Act.Abs)
                nc.scalar.activation(c["qD"][:, :w], D, Act.Square, scale=RT2)
                for bank, o in c["sp_banks"]:
                    nc.scalar.activation(c["qS"][:, o : o + H], bank[:, :],
                                         Act.Square, scale=RT2)

            def mids(c):  # DVE: thresholds (fp16 2x), extents off SP PSUM
                w = c["w"]
                nc.vector.tensor_tensor(c["mw"][:, :w], c["adx"][:, :w],
                                        c["aD"][:, :w], Op.max)
                nc.vector.tensor_tensor(c["mh"][:, :w], c["ady"][:, :w],
                                        c["aD"][:, :w], Op.max)
                for bank, o in c["sp_banks"]:
                    nc.vector.tensor_tensor(c["wr"][:, o : o + H], bank[:, :],
                                            c["mw"][:, o : o + H], Op.subtract)
                    nc.vector.tensor_tensor(c["hr"][:, o : o + H], bank[:, :],
                                            c["mh"][:, o : o + H], Op.subtract)

            def relus(c):  # ACT: clamp both extents
                w = c["w"]
                nc.scalar.activation(c["rw"][:, :w], c["wr"][:, :w], Act.Relu)
                nc.scalar.activation(c["rh"][:, :w], c["hr"][:, :w], Act.Relu)

            def ovst(c):  # Pool: ov = rw*rh
                w = c["w"]
                nc.gpsimd.tensor_tensor(c["ov"][:, :w], c["rw"][:, :w],
                                        c["rh"][:, :w], Op.mult)

            def uemm(c):  # PE: ue = eps + 2S^2 + 2D^2 - ov in PSUM
                for bank, o in c["ue_banks"]:
                    nc.tensor.matmul(bank[:, :], epsw[:, :], ones2[:, :],
                                     start=True, stop=False)
                    nc.tensor.matmul(bank[:, :], ident[:, :],
                                     c["qS"][:, o : o + H], start=False, stop=False)
                    nc.tensor.matmul(bank[:, :], ident[:, :],
                                     c["qD"][:, o : o + H], start=False, stop=False)
                    nc.tensor.matmul(bank[:, :], nident[:, :],
                                     c["ov"][:, o : o + H], start=False, stop=True)

            def recip(c):  # DVE: r = 1/(u+eps), ~18 bits, straight off PSUM
                for bank, o in c["ue_banks"]:
                    nc.vector.reciprocal_approx_fast(c["r"][:, o : o + H],
                                                     bank[:, :])

            def ioust(c):  # Pool: iou = ov * r (fp16 * fp32 -> fp16)
                w = c["w"]
                nc.gpsimd.tensor_tensor(c["iou"][:, :w], c["ov"][:, :w],
                                        c["r"][:, :w], Op.mult)

            def iou_psum(c):  # PE: per-seg partition-sums into PSUM banks
                t, w = c["t"], c["w"]
                nc.tensor.matmul(psA1[:, :], ones[:, :], c["iou"][:, :H],
                                 start=(t == 0), stop=(t == NSEG - 1))
                if w > H:
                    nc.tensor.matmul(psA2[:, :], ones[:, :], c["iou"][:, H : 2 * H],
                                     start=(t == BIG_LO), stop=(t == BIG_HI))

            def accum(c):  # ACT: loss partial rides the Ln accumulator
                t, w = c["t"], c["w"]
                nc.scalar.activation(
                    c["sc"][:, :w], c["iou"][:, :w], Act.Ln,
                    bias=eps_t[:, 0:1],
                    accum_out=accs[:, t : t + 1],
                )

            def ps_extract(bank, col, n):
                pscp = pool.tile([1, H], F32, tag="pscp", name=f"pscp_{col}")
                nc.scalar.copy(pscp[:, :n], bank[:, :n])
                nc.sync.dma_start(out=iouv_d[:, col : col + n],
                                  in_=pscp[:, :n])

            # Six-stage pipeline; per-iteration engine queue orders:
            #   ACT : ln(k-6) | rw,rh(k-3) | abs,squares(k-1)
            #   Pool: iou(k-5) | ov(k-3)
            #   PE  : iou_psum(k-6) | uemm(k-3) | drain(k) | spmm(k)
            #   DVE : mw,mh,wr,hr(k-2) | recip(k-4) | dummy,m3(k)
            # Program-order rules (shared PSUM banks, see docstring):
            #   recip(k-4) before uemm(k-3); mids(k-2) and absd(k-1)'s qS
            #   before spmm(k).
            for k in range(NSEG + 7):
                if 6 <= k <= NSEG + 5:
                    accum(C[k - 6])
                    iou_psum(C[k - 6])
                if 5 <= k <= NSEG + 4:
                    ioust(C[k - 5])
                if 4 <= k <= NSEG + 3:
                    recip(C[k - 4])
                if 3 <= k <= NSEG + 2:
                    relus(C[k - 3])
                    ovst(C[k - 3])
                    uemm(C[k - 3])
                if 2 <= k <= NSEG + 1:
                    mids(C[k - 2])
                if 1 <= k <= NSEG:
                    absd(C[k - 1])
                if k < NSEG:
                    front(k)
                    spmm(C[k])
                if k == NSEG + 5:  # psA2 closed at iter NSEG+4 (seg BIG_HI)
                    ps_extract(psA2, H, H)

            ps_extract(psA1, 0, H)
            nc.sync.dma_start(out=acc_d[:, :], in_=accs[:, :])

    if compile_passes:
        nc.compile()
    return nc


_NC_CACHE: list[bass.Bass] = []


def _get_nc() -> bass.Bass:
    if not _NC_CACHE:
        _NC_CACHE.append(_build())
    return _NC_CACHE[0]


def _run(inputs: dict, trace: bool = False, trace_kwargs: dict | None = None):
    outputs = np.ascontiguousarray(np.asarray(inputs["outputs"], dtype=np.float32))
    targets = np.ascontiguousarray(np.asarray(inputs["targets"], dtype=np.float32))
    assert outputs.shape == (N, 3) and targets.shape == (N, 3)

    nc = _get_nc()
    in_maps = [
        {
            "outputs": outputs[c * NS : (c + 1) * NS],
            "targets": targets[c * NS : (c + 1) * NS],
        }
        for c in range(NCORES)
    ]
    kw = {}
    if trace:
        kw["trace"] = True
        if trace_kwargs:
            kw["trace_kwargs"] = trace_kwargs
    res = run_bass_kernel_spmd(nc, in_maps, list(range(NCORES)), **kw)

    iou_sum = 0.0
    loss = 0.0
    for c in range(NCORES):
        acc = np.asarray(res.results[c]["acc"], dtype=np.float64)
        loss += acc.sum()
        iou_sum += np.asarray(res.results[c]["iouv"], dtype=np.float64).sum()
    loss = -loss
    return (np.float32(loss), np.float32(iou_sum)), res


def kernel(**inputs) -> tuple:
    (loss, iou_sum), _ = _run(inputs)
    return (loss, iou_sum)
